# revision 1
# baseline (speedup 1.0000x reference)
"""AttentiveItemToVec Trainium2 kernel.

Full-input contract: kernel(**inputs) takes the unsharded numpy inputs and
returns the full [512, 101, 128] float32 output. Internally shards the batch
across 8 NeuronCores (64 batches each), runs a Bass/Tile kernel per core via
run_bass_kernel_spmd, and concatenates the per-core outputs.

Per-core (64 batches): embedding rows are fetched with multi-block indirect
DMAs (4x128 rows per instruction, padded index layout so batch b's rows land
on partitions 0..100 of block b). Per batch: PE-transpose v/u to
feature-major, project (tpT/cpT with bias), squared norms via
matmul-with-ones, 1/|x| = exp(-0.5*ln(x^2)) on ScalarE (Ln/Exp/Copy/Identity
are forced into one activation table, so the kernel pays a single table
load), cosine scores, softmax without max-subtraction (cos is in [-1,1];
pad-mask enters the exp as a -1e30 bias), attention apply, output
projection. Softmax normalization and the Bc_b/R_b biases are folded into
the output stage (attention rows sum to 1).
"""

import numpy as np
from contextlib import ExitStack

# Problem constants (hardcoded per contract).
V, E, D = 100000, 128, 60
B, J, M, P = 512, 101, 50, 5120
NCORES = 8
BLOC = B // NCORES  # 64 batches per core
NEG = -1.0e30
EPS2 = 1e-12  # clamp on squared norms (eps=1e-6 on norms)

_CACHE = {}

_ACT_TABLE = "natural_log_exp_and_others"


def _patched_tables(orig_fn):
    def fn(arch):
        tabs = orig_fn(arch)
        return {
            name: (s if name == _ACT_TABLE else type(s)())
            for name, s in tabs.items()
        }
    return fn


def _build_program():
    import os
    NOPATCH = os.environ.get("K_NOPATCH") == "1"
    import concourse.bass as bass
    import concourse.tile as tile
    import concourse.bacc as bacc_mod
    from concourse import bacc, mybir

    f32 = mybir.dt.float32
    i32 = mybir.dt.int32

    nc = bacc.Bacc(
        "TRN2",
        target_bir_lowering=False,
        debug=False,
        enable_asserts=False,
    )

    temb = nc.dram_tensor("t_emb", [V, E], f32, kind="ExternalInput").ap()
    cemb = nc.dram_tensor("c_emb", [V, E], f32, kind="ExternalInput").ap()
    atwT = nc.dram_tensor("atwT", [E, D], f32, kind="ExternalInput").ap()
    acwT = nc.dram_tensor("acwT", [E, D], f32, kind="ExternalInput").ap()
    bcwT = nc.dram_tensor("bcwT", [E, E], f32, kind="ExternalInput").ap()
    rwT = nc.dram_tensor("rwT", [E, E], f32, kind="ExternalInput").ap()
    atb = nc.dram_tensor("atb", [D, 1], f32, kind="ExternalInput").ap()
    acb = nc.dram_tensor("acb", [D, 1], f32, kind="ExternalInput").ap()
    rbeff = nc.dram_tensor("rbeff", [1, E], f32, kind="ExternalInput").ap()
    eye = nc.dram_tensor("eye", [128, 128], f32, kind="ExternalInput").ap()
    eyehi = nc.dram_tensor("eyehi", [128, 64], f32, kind="ExternalInput").ap()
    offt = nc.dram_tensor("offt", [128, BLOC], i32, kind="ExternalInput").ap()
    offc = nc.dram_tensor("offc", [128, BLOC // 2], i32, kind="ExternalInput").ap()
    maskT = nc.dram_tensor("maskT", [M, BLOC], f32, kind="ExternalInput").ap()
    out = nc.dram_tensor("out", [BLOC, J, E], f32, kind="ExternalOutput").ap()

    AF = mybir.ActivationFunctionType

    with tile.TileContext(nc) as tc, ExitStack() as ctx:
        const = ctx.enter_context(tc.tile_pool(name="const", bufs=1))
        vgp = ctx.enter_context(tc.tile_pool(name="vg", bufs=BLOC // 4))
        ugp = ctx.enter_context(tc.tile_pool(name="ug", bufs=BLOC // 8))
        work = ctx.enter_context(tc.tile_pool(name="work", bufs=5))
        vecp = ctx.enter_context(tc.tile_pool(name="vec", bufs=6))
        outp = ctx.enter_context(tc.tile_pool(name="outp", bufs=3))
        psb = ctx.enter_context(tc.tile_pool(name="psb", bufs=3, space="PSUM"))
        psd = ctx.enter_context(tc.tile_pool(name="psd", bufs=3, space="PSUM"))
        psv = ctx.enter_context(tc.tile_pool(name="psv", bufs=2, space="PSUM"))

        # --- constants ---
        eye_t = const.tile([128, 128], f32)
        nc.sync.dma_start(out=eye_t[:], in_=eye[:, :])
        eyehi_t = const.tile([128, 64], f32)
        nc.sync.dma_start(out=eyehi_t[:], in_=eyehi[:, :])
        atwT_t = const.tile([E, D], f32)
        nc.sync.dma_start(out=atwT_t[:], in_=atwT[:, :])
        acwT_t = const.tile([E, D], f32)
        nc.sync.dma_start(out=acwT_t[:], in_=acwT[:, :])
        bcwT_t = const.tile([E, E], f32)
        nc.sync.dma_start(out=bcwT_t[:], in_=bcwT[:, :])
        rwT_t = const.tile([E, E], f32)
        nc.sync.dma_start(out=rwT_t[:], in_=rwT[:, :])
        atb_t = const.tile([D, 1], f32)
        nc.sync.dma_start(out=atb_t[:], in_=atb[:, :])
        acb_t = const.tile([D, 1], f32)
        nc.sync.dma_start(out=acb_t[:], in_=acb[:, :])
        rb_t = const.tile([128, E], f32)
        rb_bcast = bass.AP(tensor=rbeff.tensor, offset=0, ap=[[0, 128], [1, E]])
        nc.sync.dma_start(out=rb_t[:], in_=rb_bcast)
        offt_t = const.tile([128, BLOC], i32)
        nc.sync.dma_start(out=offt_t[:], in_=offt[:, :])
        offc_t = const.tile([128, BLOC // 2], i32)
        nc.sync.dma_start(out=offc_t[:], in_=offc[:, :])
        maskT_t = const.tile([M, BLOC], f32)
        nc.sync.dma_start(out=maskT_t[:], in_=maskT[:, :])
        ones_t = const.tile([128, 1], f32)
        nc.vector.memset(ones_t[:], 1.0)
        eps_t = const.tile([128, 1], f32)
        nc.vector.memset(eps_t[:], EPS2)

        # --- gathers: 4 blocks of 128 rows per indirect DMA instruction.
        # batch b's 101 target rows = partitions 0..100 of v block b;
        # batch b's 50 context rows = partitions 64*(b%2).. of u block b//2
        vg = [None] * (BLOC // 4)
        ug = [None] * (BLOC // 8)
        for t in range(BLOC // 8):
            for qq in (2 * t, 2 * t + 1):
                g = vgp.tile([128, 4, E], f32, tag="vg")
                for j in range(4):
                    nc.gpsimd.indirect_dma_start(
                        out=g[:, j, :],
                        out_offset=None,
                        in_=temb[:, :],
                        in_offset=bass.IndirectOffsetOnAxis(
                            ap=offt_t[:, 4 * qq + j : 4 * qq + j + 1], axis=0
                        ),
                    )
                vg[qq] = g
            g = ugp.tile([128, 4, E], f32, tag="ug")
            for j in range(4):
                nc.gpsimd.indirect_dma_start(
                    out=g[:, j, :],
                    out_offset=None,
                    in_=cemb[:, :],
                    in_offset=bass.IndirectOffsetOnAxis(
                        ap=offc_t[:, 4 * t + j : 4 * t + j + 1], axis=0
                    ),
                )
            ug[t] = g

        # --- per-batch compute, stage1/stage2 pipelined emission ---
        def stage1(b):
            v_ap = vg[b // 4][:J, b % 4, :]  # [101,128]
            ublk = b // 2
            uo = 64 * (b % 2)
            u_ap = ug[ublk // 4][uo : uo + M, ublk % 4, :]  # [50,128]

            # transposes to feature-major
            vT_ps = psb.tile([128, 128], f32, tag="pbig", space="PSUM")
            nc.tensor.transpose(out=vT_ps[:, :J], in_=v_ap, identity=eye_t[:J, :J])
            vT = work.tile([E, J], f32, tag="vT")
            nc.vector.tensor_copy(out=vT[:], in_=vT_ps[:, :J])

            uT_ps = psb.tile([128, 128], f32, tag="pbig", space="PSUM")
            u_ident = eye_t[:M, :M] if uo == 0 else eyehi_t[uo : uo + M, :M]
            nc.tensor.transpose(out=uT_ps[:, :M], in_=u_ap, identity=u_ident)
            uT = work.tile([E, M], f32, tag="uT")
            nc.scalar.copy(out=uT[:], in_=uT_ps[:, :M])

            # projections (feature-major), bias added during PSUM->SBUF copy
            tpT_ps = psb.tile([128, 128], f32, tag="pbig", space="PSUM")
            nc.tensor.matmul(
                out=tpT_ps[:D, :J], lhsT=atwT_t[:], rhs=vT[:], start=True, stop=True
            )
            tpT = work.tile([D, J], f32, tag="tpT")
            nc.scalar.activation(
                out=tpT[:], in_=tpT_ps[:D, :J], func=AF.Identity, bias=atb_t[:], scale=1.0
            )

            cpT_ps = psb.tile([128, 128], f32, tag="pbig", space="PSUM")
            nc.tensor.matmul(
                out=cpT_ps[:D, :M], lhsT=acwT_t[:], rhs=uT[:], start=True, stop=True
            )
            cpT = work.tile([D, M], f32, tag="cpT")
            nc.scalar.activation(
                out=cpT[:], in_=cpT_ps[:D, :M], func=AF.Identity, bias=acb_t[:], scale=1.0
            )

            # squared norms via matmul-with-ones -> column vectors
            tpT2 = work.tile([D, J], f32, tag="tpT2")
            nc.vector.tensor_mul(out=tpT2[:], in0=tpT[:], in1=tpT[:])
            cpT2 = work.tile([D, M], f32, tag="cpT2")
            nc.vector.tensor_mul(out=cpT2[:], in0=cpT[:], in1=cpT[:])

            nt2_ps = psv.tile([128, 1], f32, tag="pvec", space="PSUM")
            nc.tensor.matmul(
                out=nt2_ps[:J, :], lhsT=tpT2[:], rhs=ones_t[:D, :], start=True, stop=True
            )
            nc2_ps = psv.tile([128, 1], f32, tag="pvec", space="PSUM")
            nc.tensor.matmul(
                out=nc2_ps[:M, :], lhsT=cpT2[:], rhs=ones_t[:D, :], start=True, stop=True
            )

            # 1/|x| = exp(-0.5 * ln(x^2 + eps)) -- Ln and Exp share one table
            lnt = vecp.tile([128, 1], f32, tag="lnt")
            nc.scalar.activation(
                out=lnt[:J], in_=nt2_ps[:J, :], func=AF.Ln, bias=eps_t[:J, :]
            )
            ntinv = vecp.tile([128, 1], f32, tag="ntinv")
            nc.scalar.activation(
                out=ntinv[:J], in_=lnt[:J], func=AF.Exp, scale=-0.5
            )

            lnc = vecp.tile([128, 1], f32, tag="lnc")
            nc.scalar.activation(
                out=lnc[:M], in_=nc2_ps[:M, :], func=AF.Ln, bias=eps_t[:M, :]
            )
            ncinv = vecp.tile([128, 1], f32, tag="ncinv")
            nc.scalar.activation(
                out=ncinv[:M], in_=lnc[:M], func=AF.Exp, scale=-0.5
            )

            # dot products (own double-buffered bank; spans into stage2)
            dot_ps = psd.tile([128, 128], f32, tag="pdot", space="PSUM")
            nc.tensor.matmul(
                out=dot_ps[:J, :M], lhsT=tpT[:], rhs=cpT[:], start=True, stop=True
            )
            return dot_ps, ntinv, ncinv, uT

        def stage2(b, st):
            dot_ps, ntinv, ncinv, uT = st
            dotn = work.tile([J, M], f32, tag="dotn")
            nc.vector.tensor_scalar_mul(dotn[:], dot_ps[:J, :M], ntinv[:J, :])

            # transpose to [50,101]; exp(ncinv*x + mask) in one activation
            dotT_ps = psb.tile([128, 128], f32, tag="pbig", space="PSUM")
            nc.tensor.transpose(
                out=dotT_ps[:M, :J], in_=dotn[:], identity=eye_t[:J, :J]
            )
            attnT = work.tile([M, J], f32, tag="attnT")
            nc.scalar.activation(
                out=attnT[:],
                in_=dotT_ps[:M, :J],
                func=AF.Exp,
                bias=maskT_t[:, b : b + 1],
                scale=ncinv[:M, :],
            )

            # softmax denominators (per target row j)
            cs_ps = psv.tile([128, 1], f32, tag="pvec", space="PSUM")
            nc.tensor.matmul(
                out=cs_ps[:J, :], lhsT=attnT[:], rhs=ones_t[:M, :], start=True, stop=True
            )
            rsinv = vecp.tile([128, 1], f32, tag="rsinv")
            nc.vector.reciprocal(out=rsinv[:J], in_=cs_ps[:J, :1])


            # bu = u @ Bc_w.T (bias folded into rbeff), then alphaT, then output

            bu_ps = psb.tile([128, 128], f32, tag="pbig", space="PSUM")
            nc.tensor.matmul(
                out=bu_ps[:M, :E], lhsT=uT[:], rhs=bcwT_t[:], start=True, stop=True
            )
            bu = work.tile([M, E], f32, tag="bu")
            nc.scalar.copy(out=bu[:], in_=bu_ps[:M, :])

            al_ps = psb.tile([128, 128], f32, tag="pbig", space="PSUM")
            nc.tensor.matmul(
                out=al_ps[:E, :J], lhsT=bu[:], rhs=attnT[:], start=True, stop=True
            )
            alT = work.tile([E, J], f32, tag="alT")
            nc.vector.tensor_copy(out=alT[:], in_=al_ps[:, :J])

            o_ps = psb.tile([128, 128], f32, tag="pbig", space="PSUM")
            nc.tensor.matmul(
                out=o_ps[:J, :E], lhsT=alT[:], rhs=rwT_t[:], start=True, stop=True
            )
            o_sb = outp.tile([J, E], f32, tag="o")
            nc.vector.scalar_tensor_tensor(
                out=o_sb[:], in0=o_ps[:J, :E], scalar=rsinv[:J, :],
                in1=rb_t[:J, :], op0=mybir.AluOpType.mult,
                op1=mybir.AluOpType.add,
            )
            nc.sync.dma_start(out=out[b], in_=o_sb[:])

        # per-engine execution is in program order: interleave batch b+1's
        # stage1 with batch b's stage2 so independent work hides the waits
        LAG = 2
        pend = [(0, stage1(0))]
        for b in range(1, BLOC):
            pend.append((b, stage1(b)))
            if len(pend) > LAG:
                stage2(*pend.pop(0))
        while pend:
            stage2(*pend.pop(0))

    # Force every activation onto the one table holding Ln/Exp/Copy/Identity
    # so the kernel pays a single table load. Indices into act_info.json are
    # preserved (other sets are just emptied for the placement pass), so the
    # runtime table mapping stays correct.
    if NOPATCH:
        nc.compile()
    else:
        orig = bacc_mod.get_activation_tables
        bacc_mod.get_activation_tables = _patched_tables(orig)
        try:
            nc.compile()
        finally:
            bacc_mod.get_activation_tables = orig
    return nc


def _get_program():
    if "nc" not in _CACHE:
        _CACHE["nc"] = _build_program()
    return _CACHE["nc"]


def _prep_inputs(batch_titems, batch_citems, batch_pad_ids, t_emb, c_emb,
                 Ac_w, Ac_b, At_w, At_b, Bc_w, Bc_b, R_w, R_b):
    f = lambda x: np.ascontiguousarray(np.asarray(x, dtype=np.float32))
    t_emb = f(t_emb)
    c_emb = f(c_emb)
    tit = np.asarray(batch_titems).astype(np.int32)
    cit = np.asarray(batch_citems).astype(np.int32)
    pad = np.asarray(batch_pad_ids).astype(np.int64)

    mask = np.zeros((B, M), np.float32)
    mask[pad[0], pad[1]] = NEG

    atwT = f(np.asarray(At_w).T)
    acwT = f(np.asarray(Ac_w).T)
    bcwT = f(np.asarray(Bc_w).T)
    rwT = f(np.asarray(R_w).T)
    atb = f(np.asarray(At_b).reshape(D, 1))
    acb = f(np.asarray(Ac_b).reshape(D, 1))
    rbeff = f(
        (np.asarray(R_b, np.float32)
         + np.asarray(R_w, np.float32) @ np.asarray(Bc_b, np.float32)).reshape(1, E)
    )
    eye = np.eye(128, dtype=np.float32)
    eyehi = np.zeros((128, 64), np.float32)
    eyehi[64:, :] = np.eye(64, dtype=np.float32)

    in_maps = []
    for c in range(NCORES):
        s = c * BLOC
        tslice = tit[s : s + BLOC]  # [64,101]
        tpad = np.zeros((BLOC, 128), np.int32)
        tpad[:, :J] = tslice
        offt = np.ascontiguousarray(tpad.reshape(-1).reshape(BLOC, 128).T)

        cslice = cit[s : s + BLOC]  # [64,50]
        cpad = np.zeros((BLOC, 64), np.int32)
        cpad[:, :M] = cslice
        offc = np.ascontiguousarray(cpad.reshape(-1).reshape(BLOC // 2, 128).T)

        maskTc = np.ascontiguousarray(mask[s : s + BLOC].T)  # [50,64]

        in_maps.append(
            {
                "t_emb": t_emb,
                "c_emb": c_emb,
                "atwT": atwT,
                "acwT": acwT,
                "bcwT": bcwT,
                "rwT": rwT,
                "atb": atb,
                "acb": acb,
                "rbeff": rbeff,
                "eye": eye,
                "eyehi": eyehi,
                "offt": offt,
                "offc": offc,
                "maskT": maskTc,
            }
        )
    return in_maps


def run_sharded(in_maps, **kwargs):
    from concourse.bass_utils import run_bass_kernel_spmd

    nc = _get_program()
    res = run_bass_kernel_spmd(nc, in_maps, core_ids=list(range(NCORES)), **kwargs)
    outs = [res.results[c]["out"] for c in range(NCORES)]
    full = np.concatenate(outs, axis=0)
    return full, res


def kernel(**inputs):
    in_maps = _prep_inputs(**inputs)
    full, _ = run_sharded(in_maps)
    return full.astype(np.float32)



# revision 39
# speedup vs baseline: 1.7827x; 1.7827x over previous
"""AttentiveItemToVec Trainium2 kernel (v2).

Full-input contract: kernel(**inputs) takes the unsharded numpy inputs and
returns the full [512, 101, 128] float32 output. Internally shards the batch
across 8 NeuronCores (64 batches each), runs a Bass/Tile kernel per core via
run_bass_kernel_spmd, and concatenates the per-core outputs.

v2 design (per core, 64 batches):
- Embedding tables converted to bf16 on host; 12 wide multi-row indirect
  DMAs (1024 rows each) amortize the ~1us SWDGE fixed cost per gather.
- All PE work in bf16 (1 cyc/row) with fp32 PSUM accumulation.
- v rows padded to 128/batch, u rows padded to 64/batch (2 batches per
  128-partition block) so every matmul operand sits at base partition 0/64.
- Projections computed feature-major in [*, 404/512]-wide banks, two
  D=60(+4 pad) blocks stacked at partitions 0/64 per bank; squared norms via
  one Act Square per bank + per-batch ones-matmuls into a persistent PSUM
  vector bank; 1/norm = exp(-0.5*ln(x+eps)) on two whole-bank Act ops.
- attn = exp applied m-major per batch pair ([0:50] and [64:114] rows of one
  bank; the [50:64] gap memset once per rotating buffer) with per-partition
  ncinv scale + additive -1e30 pad mask; softmax denominators via
  ones-matmul columns; normalization folded into the output stage.
- (R_w @ Bc_w) precomputed on host fuses the Bc/R projections into one
  matmul; Bc_b/R_b land in a host-side rbeff row added after the device run.
- Wide DVE ops use free-dim-stride-0 broadcast APs (ntinv over 50 cols,
  rsinv over 128 cols) so one op covers 8 resp. 4 batches.
"""

import numpy as np
from contextlib import ExitStack

V, E, D = 100000, 128, 60
B, J, M, P = 512, 101, 50, 5120
NCORES = 8
BLOC = B // NCORES  # 64
NEG = -1.0e30
EPS2 = 1e-12

NVG = 8   # v chunks (8 batches each)
NVTAB = 8192   # per-core deduped t_emb shard rows (>= 64*101 padded uniques)
NUTAB = 4096   # per-core deduped c_emb shard rows

VA = (0, 1, 2, 3)
VB = (4, 5, 6, 7)

_CACHE = {}

_ACT_TABLE = "natural_log_exp_and_others"


def _patched_tables(orig_fn):
    def fn(arch):
        tabs = orig_fn(arch)
        return {
            name: (s if name == _ACT_TABLE else type(s)())
            for name, s in tabs.items()
        }
    return fn


_CFG = {
    "copy_mod": 3,       # 1-in-N copies go to DVE
    "cp_tsadd_act": 0,   # cp bias-copy on Act
    "trb": 3, "pdot": 2, "pout": 2,
    "stt_pool": 0,
    "sq_pool": 0,
    "trunc": 99,       # cos-scale on gpsimd
}


def _build_program():
    import os
    NOPATCH = os.environ.get("K_NOPATCH") == "1"
    import concourse.bass as bass
    import concourse.tile as tile
    import concourse.bacc as bacc_mod
    from concourse import bacc, mybir

    f32 = mybir.dt.float32
    bf16 = mybir.dt.bfloat16
    i32 = mybir.dt.int32
    i16 = mybir.dt.int16
    AF = mybir.ActivationFunctionType
    MUL = mybir.AluOpType.mult

    nc = bacc.Bacc(
        "TRN2",
        target_bir_lowering=False,
        debug=False,
        enable_asserts=False,
    )

    temb = nc.dram_tensor("temb", [V, E], bf16, kind="ExternalInput").ap()
    cemb = nc.dram_tensor("cemb", [V, E], bf16, kind="ExternalInput").ap()
    atw = nc.dram_tensor("atw", [128, 64], bf16, kind="ExternalInput").ap()
    acw = nc.dram_tensor("acw", [128, 64], bf16, kind="ExternalInput").ap()
    w2T = nc.dram_tensor("w2T", [128, 128], bf16, kind="ExternalInput").ap()
    atb = nc.dram_tensor("atb", [128, 1], f32, kind="ExternalInput").ap()
    acb = nc.dram_tensor("acb", [128, 1], f32, kind="ExternalInput").ap()
    eye = nc.dram_tensor("eye", [128, 128], bf16, kind="ExternalInput").ap()
    offt = nc.dram_tensor("offt", [128, BLOC], i32, kind="ExternalInput").ap()
    offc = nc.dram_tensor("offc", [128, BLOC // 2], i32, kind="ExternalInput").ap()
    maskT = nc.dram_tensor("maskT", [128, BLOC], f32, kind="ExternalInput").ap()
    out = nc.dram_tensor("out", [J, BLOC, E], bf16, kind="ExternalOutput").ap()

    with tile.TileContext(nc) as tc, ExitStack() as ctx:
        const = ctx.enter_context(tc.tile_pool(name="const", bufs=1))
        vgp = ctx.enter_context(tc.tile_pool(name="vg", bufs=4))
        ugp = ctx.enter_context(tc.tile_pool(name="ug", bufs=2))
        uTp = ctx.enter_context(tc.tile_pool(name="uT", bufs=8))
        vTp = ctx.enter_context(tc.tile_pool(name="vT", bufs=16))
        tpsp = ctx.enter_context(tc.tile_pool(name="tps", bufs=8))
        tp2p = ctx.enter_context(tc.tile_pool(name="tp2", bufs=8))
        cpsp = ctx.enter_context(tc.tile_pool(name="cps", bufs=4))
        cp2p = ctx.enter_context(tc.tile_pool(name="cp2", bufs=4))
        burp = ctx.enter_context(tc.tile_pool(name="bur", bufs=16))
        cosp = ctx.enter_context(tc.tile_pool(name="cos", bufs=3))
        attp = ctx.enter_context(tc.tile_pool(name="att", bufs=18))
        nrmp = ctx.enter_context(tc.tile_pool(name="nrm", bufs=1))
        osbp = ctx.enter_context(tc.tile_pool(name="osb", bufs=4))
        ptr = ctx.enter_context(tc.tile_pool(name="ptr", bufs=_CFG["trb"], space="PSUM"))
        pdot = ctx.enter_context(tc.tile_pool(name="pdot", bufs=_CFG["pdot"], space="PSUM"))
        pout = ctx.enter_context(tc.tile_pool(name="pout", bufs=_CFG["pout"], space="PSUM"))
        pvp = ctx.enter_context(tc.tile_pool(name="pv", bufs=1, space="PSUM"))

        # --- offset tiles first, so the gathers launch before other consts ---
        offc_t = const.tile([128, BLOC // 2], i32)
        nc.sync.dma_start(out=offc_t[:], in_=offc[:, :])
        offt_t = const.tile([128, BLOC], i32)
        nc.sync.dma_start(out=offt_t[:], in_=offt[:, :])

        # --- gathers: 1024 rows per indirect DMA ---
        ug = []
        for g in range(4):
            t = ugp.tile([128, 8, E], bf16, tag="ug")
            for j in range(8):
                nc.gpsimd.indirect_dma_start(
                    out=t[:, j, :], out_offset=None, in_=cemb[:, :],
                    in_offset=bass.IndirectOffsetOnAxis(
                        ap=offc_t[:, 8 * g + j : 8 * g + j + 1], axis=0))
            ug.append(t)
        vg = [None] * 4

        def v_gather(q):
            t = vgp.tile([128, 16, E], bf16, tag="vg")
            for j in range(16):
                nc.gpsimd.indirect_dma_start(
                    out=t[:, j, :], out_offset=None, in_=temb[:, :],
                    in_offset=bass.IndirectOffsetOnAxis(
                        ap=offt_t[:, 16 * q + j : 16 * q + j + 1], axis=0))
            vg[q] = t

        for q in range(3):
            v_gather(q)

        # --- remaining constants (overlap with the gathers) ---
        eye_t = const.tile([128, 128], bf16)
        nc.sync.dma_start(out=eye_t[:], in_=eye[:, :])
        atw_t = const.tile([128, 64], bf16)
        nc.sync.dma_start(out=atw_t[:], in_=atw[:, :])
        acw_t = const.tile([128, 64], bf16)
        nc.sync.dma_start(out=acw_t[:], in_=acw[:, :])
        w2T_t = const.tile([128, 128], bf16)
        nc.sync.dma_start(out=w2T_t[:], in_=w2T[:, :])
        atb_t = const.tile([128, 1], f32)
        nc.sync.dma_start(out=atb_t[:], in_=atb[:, :])
        acb_t = const.tile([128, 1], f32)
        nc.sync.dma_start(out=acb_t[:], in_=acb[:, :])
        maskT_t = const.tile([128, BLOC], f32)
        nc.sync.dma_start(out=maskT_t[:], in_=maskT[:, :])
        ones_bf = const.tile([128, 1], bf16)
        nc.vector.memset(ones_bf[:], 1.0)
        eps_t = const.tile([128, 1], f32)
        nc.vector.memset(eps_t[:], EPS2)

        # persistent PSUM vector bank: cols 0:64 nt2, 64:96 nc2, 96:160 rsum
        pvec = pvp.tile([128, 192], f32, tag="vec", space="PSUM")

        alt = [0]  # copy-engine alternator

        def copy_out(dst_ap, src_ap):
            if alt[0] % _CFG["copy_mod"] == 0:
                nc.vector.tensor_copy(out=dst_ap, in_=src_ap)
            else:
                nc.scalar.copy(out=dst_ap, in_=src_ap)
            alt[0] += 1

        # ---- u / cp / buR phase (per 16-batch chunk), software-pipelined ----
        uT = [None] * 8
        cps = [None] * 4
        cp2 = [None] * 4
        bur = [None] * 16

        def u_transp(c):
            for half in range(2):
                trb = ptr.tile([128, 512], bf16, tag="trb", space="PSUM")
                for q in range(4):
                    j = 4 * half + q
                    blk = 8 * c + j
                    nc.tensor.transpose(
                        out=trb[:, 128 * q : 128 * q + 128],
                        in_=ug[blk // 8][:, blk % 8, :], identity=eye_t[:, :])
                t = uTp.tile([128, 512], bf16, tag="uT")
                copy_out(t[:], trb[:, :])
                uT[2 * c + half] = t

        def u_work(c):
            # projections: 8 pairs -> 2 banks (rows 0:64 only), cps [64, 1024]
            cs = cpsp.tile([64, 1024], bf16, tag="cps")
            for h2 in range(2):
                pj = pdot.tile([128, 512], f32, tag="pd", space="PSUM")
                for s in range(4):
                    tt = 4 * h2 + s
                    pack = 2 * c + tt // 4
                    q = tt % 4
                    nc.tensor.matmul(
                        out=pj[0:64, 128 * s : 128 * s + 128],
                        lhsT=acw_t[:], rhs=uT[pack][:, 128 * q : 128 * q + 128],
                        start=True, stop=True)
                nc.vector.tensor_scalar_add(
                    cs[:, 512 * h2 : 512 * h2 + 512], pj[0:64, 0:512], acb_t[0:64, :])
            cps[c] = cs
            sq = cp2p.tile([64, 1024], bf16, tag="cp2")
            nc.vector.tensor_mul(out=sq[:], in0=cs[:], in1=cs[:])
            cp2[c] = sq
            # buR: one [64, 128] block per batch, 4 batches per bank
            for w in range(4):
                burb = pout.tile([128, 512], f32, tag="po", space="PSUM")
                for s3 in range(4):
                    bb16 = 4 * w + s3          # batch within chunk (0..15)
                    tt = bb16 // 2
                    eo = bb16 % 2
                    pack = 2 * c + tt // 4
                    q = tt % 4
                    nc.tensor.matmul(
                        out=burb[0:64, 128 * s3 : 128 * s3 + 128],
                        lhsT=uT[pack][:, 128 * q + 64 * eo : 128 * q + 64 * eo + 64],
                        rhs=w2T_t[:], start=True, stop=True)
                bt = burp.tile([64, 512], bf16, tag="bur")
                copy_out(bt[:], burb[0:64, :])
                bur[4 * c + w] = bt

        TR = _CFG["trunc"]
        TR = _CFG["trunc"]
        for c in range(4):
            if TR >= 2:
                u_transp(c)
            if c >= 1 and TR >= 3:
                u_work(c - 1)
        if TR >= 3:
            u_work(3)

        # nc2 norm matmuls (feed only the global Ln below): col = batch
        for c in range(4 if TR >= 3 else 0):
            sq = cp2[c]
            for bb16 in range(16):
                b = 16 * c + bb16
                nc.tensor.matmul(
                    out=pvec[0:64, 64 + b : 65 + b],
                    lhsT=sq[0:64, 64 * bb16 : 64 * bb16 + 64],
                    rhs=ones_bf[0:64, :], start=True, stop=True)

        if TR < 3:
            nc.compile if False else None
        lnc = nrmp.tile([64, 64], f32, tag="lnc")
        ncinv = nrmp.tile([64, 64], f32, tag="ncinv")
        if TR >= 3:
            nc.scalar.activation(out=lnc[:], in_=pvec[0:64, 64:128], func=AF.Ln,
                                 bias=eps_t[0:64, :])
            nc.scalar.activation(out=ncinv[:], in_=lnc[:], func=AF.Exp, scale=-0.5)

        # ---- v phase (per 8-batch chunk), transposes run one chunk ahead ----
        tps = [None] * 8
        vTs = [None] * 8
        tp2s = [None] * 8

        def v_transp(k):
            vT2 = []
            for half, idxs in ((0, VA), (1, VB)):
                trb = ptr.tile([128, 512], bf16, tag="trb", space="PSUM")
                for i, boff in enumerate(idxs):
                    b = 8 * k + boff
                    nc.tensor.transpose(
                        out=trb[:, 102 * i : 102 * i + J],
                        in_=vg[b // 16][0:J, b % 16, :], identity=eye_t[0:J, 0:J])
                t = vTp.tile([128, 404], bf16, tag="vT")
                tr_ap = trb[:, 0:512]
                src = bass.AP(tensor=tr_ap.tensor, offset=tr_ap.offset,
                              ap=[tr_ap.ap[0], [102, 4], [1, J]])
                t_ap = t[:]
                dst = bass.AP(tensor=t_ap.tensor, offset=t_ap.offset,
                              ap=[t_ap.ap[0], [J, 4], [1, J]])
                copy_out(dst, src)
                vT2.append(t)
            vTs[k] = vT2

        def v_proj(k):
            vT2 = vTs[k]
            ts = tpsp.tile([64, 808], bf16, tag="tps")
            for h2 in range(2):
                projb = pdot.tile([128, 512], f32, tag="pd", space="PSUM")
                nc.tensor.matmul(out=projb[0:64, 0:404], lhsT=atw_t[:],
                                 rhs=vT2[h2][:], start=True, stop=True)
                nc.vector.tensor_scalar_add(
                    ts[:, 404 * h2 : 404 * h2 + 404], projb[0:64, 0:404], atb_t[0:64, :])
            tps[k] = ts
            sq = tp2p.tile([64, 808], bf16, tag="tp2")
            nc.vector.tensor_mul(out=sq[:], in0=ts[:], in1=ts[:])
            tp2s[k] = sq

        lnt = nrmp.tile([J, 64], f32, tag="lnt")
        ntinv = nrmp.tile([J, 64], f32, tag="ntinv")
        rsinv = nrmp.tile([J, 64], f32, tag="rsinv")

        def v_norms(kk):
            sq = tp2s[kk]
            for boff in range(8):
                b = 8 * kk + boff
                nc.tensor.matmul(
                    out=pvec[0:J, b : b + 1],
                    lhsT=sq[0:64, 101 * boff : 101 * boff + 101],
                    rhs=ones_bf[0:64, :], start=True, stop=True)

        def v_ntinv(kk):
            sl = slice(8 * kk, 8 * kk + 8)
            nc.scalar.activation(out=lnt[0:J, sl], in_=pvec[0:J, sl],
                                 func=AF.Ln, bias=eps_t[0:J, :])
            nc.scalar.activation(out=ntinv[0:J, sl], in_=lnt[0:J, sl],
                                 func=AF.Exp, scale=-0.5)

        # ---- attention stream (per 8-batch group), dots run one group ahead ----
        IA = {0: 0, 1: 1, 4: 2, 5: 3}
        IB = {2: 0, 3: 1, 6: 2, 7: 3}

        def attn_dots(g):
            dotb = pdot.tile([128, 512], f32, tag="pd", space="PSUM")
            for bb in range(_CFG.get("ndots", 8)):
                b = 8 * g + bb
                tt = (b % 16) // 2
                nc.tensor.matmul(
                    out=dotb[0:J, 64 * bb : 64 * bb + 64],
                    lhsT=tps[g][0:64, 101 * bb : 101 * bb + 101],
                    rhs=cps[b // 16][0:64,
                                     128 * tt + 64 * (b % 2) :
                                     128 * tt + 64 * (b % 2) + 64],
                    start=True, stop=True)
            cosg = cosp.tile([J, 512], bf16, tag="cos")
            if _CFG.get("no_stt"):
                nc.vector.memset(cosg[:], 0.0)
                return cosg
            nt_sl = ntinv[0:J, 8 * g : 8 * g + 8]
            in1 = bass.AP(tensor=nt_sl.tensor, offset=nt_sl.offset,
                          ap=[nt_sl.ap[0], nt_sl.ap[1], [0, 64]])
            eng = nc.gpsimd if _CFG["stt_pool"] else nc.vector
            if _CFG.get("plain_stt"):
                eng.tensor_tensor(out=cosg[:], in0=dotb[0:J, 0:512],
                                  in1=dotb[0:J, 0:512], op=MUL)
            else:
                eng.tensor_tensor(out=cosg[:], in0=dotb[0:J, 0:512], in1=in1, op=MUL)
            return cosg

        def attn_ab(g, cosg):
            ats = []
            for bb in range(8):
                ab = ptr.tile([128, 512], bf16, tag="trb", space="PSUM")
                nc.tensor.transpose(
                    out=ab[0:64, 0:J], in_=cosg[:, 64 * bb : 64 * bb + 64],
                    identity=eye_t[0:J, 0:J])
                ats.append(ab)
            for bb in range(8):
                b = 8 * g + bb
                at = attp.tile([64, J], bf16, tag="att")
                nc.scalar.activation(
                    out=at[:, :], in_=ats[bb][0:64, 0:J], func=AF.Exp,
                    scale=ncinv[:, b : b + 1],
                    bias=maskT_t[0:64, b : b + 1])
                ats[bb] = at
            return ats

        def attn_cd(g, ats):
            for bb in range(8):
                b = 8 * g + bb
                nc.tensor.matmul(
                    out=pvec[0:J, 128 + b : 129 + b],
                    lhsT=ats[bb][0:50, :], rhs=ones_bf[0:50, :],
                    start=True, stop=True)

            nc.vector.reciprocal(out=rsinv[0:J, 8 * g : 8 * g + 8],
                                 in_=pvec[0:J, 128 + 8 * g : 136 + 8 * g])

            for ob in range(2):
                outb = pout.tile([128, 512], f32, tag="po", space="PSUM")
                for bb4 in range(4):
                    b = 8 * g + 4 * ob + bb4
                    nc.tensor.matmul(
                        out=outb[0:J, 128 * bb4 : 128 * bb4 + 128],
                        lhsT=ats[4 * ob + bb4][0:50, :],
                        rhs=bur[b // 4][0:50, 128 * (b % 4) : 128 * (b % 4) + 128],
                        start=True, stop=True)
                osbt = osbp.tile([J, 4, E], bf16, tag="osb")
                ob_ap = outb[0:J, 0:512]
                in0 = bass.AP(tensor=ob_ap.tensor, offset=ob_ap.offset,
                              ap=[ob_ap.ap[0], [128, 4], [1, 128]])
                rs_sl = rsinv[0:J, 8 * g + 4 * ob : 8 * g + 4 * ob + 4]
                in1 = bass.AP(tensor=rs_sl.tensor, offset=rs_sl.offset,
                              ap=[rs_sl.ap[0], rs_sl.ap[1], [0, 128]])
                nc.vector.tensor_tensor(out=osbt[:], in0=in0, in1=in1, op=MUL)
                b0 = 8 * g + 4 * ob
                dst = bass.AP(tensor=out.tensor, offset=b0 * E,
                              ap=[[BLOC * E, J], [E, 4], [1, E]])
                nc.sync.dma_start(out=dst, in_=osbt[:])

        # unified loop: v chunks and attention stages interleaved so the
        # stream starts as soon as chunk 0 is projected; late v gathers are
        # emitted mid-loop so Pool can alternate gathers with cos-scales
        cos_store = {}
        ats_store = {}
        for k in range(NVG):
            if TR >= 4:
                v_transp(k)
                if k >= 1:
                    v_proj(k - 1)
                    v_norms(k - 1)
                    v_ntinv(k - 1)
            if k == 2:
                v_gather(3)
            if k >= 2 and TR >= 5:
                cos_store[k - 2] = attn_dots(k - 2)
            if k >= 3 and TR >= 6:
                ats_store[k - 3] = attn_ab(k - 3, cos_store.pop(k - 3))
            if k >= 4 and TR >= 7:
                attn_cd(k - 4, ats_store.pop(k - 4))
        if TR >= 4:
            v_proj(7)
            v_norms(7)
            v_ntinv(7)
        if TR >= 5:
            cos_store[6] = attn_dots(6)
        if TR >= 6:
            ats_store[5] = attn_ab(5, cos_store.pop(5))
        if TR >= 7:
            attn_cd(4, ats_store.pop(4))
        if TR >= 5:
            cos_store[7] = attn_dots(7)
        if TR >= 6:
            ats_store[6] = attn_ab(6, cos_store.pop(6))
        if TR >= 7:
            attn_cd(5, ats_store.pop(5))
        if TR >= 6:
            ats_store[7] = attn_ab(7, cos_store.pop(7))
        if TR >= 7:
            attn_cd(6, ats_store.pop(6))
            attn_cd(7, ats_store.pop(7))

    if NOPATCH:
        nc.compile()
    else:
        orig = bacc_mod.get_activation_tables
        bacc_mod.get_activation_tables = _patched_tables(orig)
        try:
            nc.compile()
        finally:
            bacc_mod.get_activation_tables = orig
    return nc


def _get_program():
    if "nc" not in _CACHE:
        _CACHE["nc"] = _build_program()
    return _CACHE["nc"]


def _prep_inputs(batch_titems, batch_citems, batch_pad_ids, t_emb, c_emb,
                 Ac_w, Ac_b, At_w, At_b, Bc_w, Bc_b, R_w, R_b):
    import ml_dtypes
    bf = ml_dtypes.bfloat16
    f = lambda x: np.ascontiguousarray(np.asarray(x, dtype=np.float32))
    temb = np.ascontiguousarray(np.asarray(t_emb, np.float32).astype(bf))
    cemb = np.ascontiguousarray(np.asarray(c_emb, np.float32).astype(bf))
    tit = np.asarray(batch_titems).astype(np.int32)
    cit = np.asarray(batch_citems).astype(np.int32)
    pad = np.asarray(batch_pad_ids).astype(np.int64)

    mask = np.zeros((B, M), np.float32)
    mask[pad[0], pad[1]] = NEG

    At_w = f(At_w); Ac_w = f(Ac_w); Bc_w = f(Bc_w); R_w = f(R_w)
    At_b = f(At_b); Ac_b = f(Ac_b); Bc_b = f(Bc_b); R_b = f(R_b)

    atw = np.zeros((128, 64), np.float32)
    atw[:, 0:D] = At_w.T
    acw = np.zeros((128, 64), np.float32)
    acw[:, 0:D] = Ac_w.T
    w2T = np.ascontiguousarray((R_w @ Bc_w).T)

    atb = np.zeros((128, 1), np.float32)
    atb[0:D, 0] = At_b
    atb[64:64 + D, 0] = At_b
    acb = np.zeros((128, 1), np.float32)
    acb[0:D, 0] = Ac_b
    acb[64:64 + D, 0] = Ac_b

    rbeff = (R_b + R_w @ Bc_b).astype(np.float32)

    eye = np.eye(128, dtype=np.float32)

    atw_b = atw.astype(bf); acw_b = acw.astype(bf)
    w2T_b = w2T.astype(bf); eye_b = eye.astype(bf)

    in_maps = []
    for c in range(NCORES):
        s = c * BLOC
        # v offsets: [128, 64], col b = batch b, partitions 0:101 = its rows
        offt = np.zeros((128, BLOC), np.int32)
        offt[0:J, :] = tit[s : s + BLOC].T
        # u offsets: [128, 32], col t = pair (2t, 2t+1) at partition bases 0/64
        offc = np.zeros((128, BLOC // 2), np.int32)
        cslice = cit[s : s + BLOC]  # [64, 50]
        offc[0:M, :] = cslice[0::2].T
        offc[64:64 + M, :] = cslice[1::2].T
        # mask: [128, 64] col b rows 0:50 = mask[b], rows 50:64 = NEG (pads)
        mk = np.full((128, BLOC), NEG, np.float32)
        mk[0:M, :] = mask[s : s + BLOC].T

        in_maps.append({
            "temb": temb, "cemb": cemb,
            "atw": atw_b, "acw": acw_b, "w2T": w2T_b,
            "atb": atb, "acb": acb, "eye": eye_b,
            "offt": offt, "offc": offc, "maskT": mk,
        })
    return in_maps, rbeff


def run_sharded(in_maps, **kwargs):
    from concourse.bass_utils import run_bass_kernel_spmd

    nc = _get_program()
    res = run_bass_kernel_spmd(nc, in_maps, core_ids=list(range(NCORES)), **kwargs)
    outs = [np.asarray(res.results[c]["out"]).transpose(1, 0, 2).astype(np.float32)
            for c in range(NCORES)]
    full = np.concatenate(outs, axis=0)
    return full, res


def kernel(**inputs):
    in_maps, rbeff = _prep_inputs(**inputs)
    full, _ = run_sharded(in_maps)
    return (full + rbeff[None, None, :]).astype(np.float32)


# revision 45
# speedup vs baseline: 1.8305x; 1.0268x over previous
"""AttentiveItemToVec Trainium2 kernel (v2).

Full-input contract: kernel(**inputs) takes the unsharded numpy inputs and
returns the full [512, 101, 128] float32 output. Internally shards the batch
across 8 NeuronCores (64 batches each), runs a Bass/Tile kernel per core via
run_bass_kernel_spmd, and concatenates the per-core outputs.

v2 design (per core, 64 batches):
- Embedding tables converted to bf16 on host; 12 wide multi-row indirect
  DMAs (1024 rows each) amortize the ~1us SWDGE fixed cost per gather.
- All PE work in bf16 (1 cyc/row) with fp32 PSUM accumulation.
- v rows padded to 128/batch, u rows padded to 64/batch (2 batches per
  128-partition block) so every matmul operand sits at base partition 0/64.
- Projections computed feature-major in [*, 404/512]-wide banks, two
  D=60(+4 pad) blocks stacked at partitions 0/64 per bank; squared norms via
  one Act Square per bank + per-batch ones-matmuls into a persistent PSUM
  vector bank; 1/norm = exp(-0.5*ln(x+eps)) on two whole-bank Act ops.
- attn = exp applied m-major per batch pair ([0:50] and [64:114] rows of one
  bank; the [50:64] gap memset once per rotating buffer) with per-partition
  ncinv scale + additive -1e30 pad mask; softmax denominators via
  ones-matmul columns; normalization folded into the output stage.
- (R_w @ Bc_w) precomputed on host fuses the Bc/R projections into one
  matmul; Bc_b/R_b land in a host-side rbeff row added after the device run.
- Wide DVE ops use free-dim-stride-0 broadcast APs (ntinv over 50 cols,
  rsinv over 128 cols) so one op covers 8 resp. 4 batches.
"""

import numpy as np
from contextlib import ExitStack

V, E, D = 100000, 128, 60
B, J, M, P = 512, 101, 50, 5120
NCORES = 8
BLOC = B // NCORES  # 64
NEG = -1.0e30
EPS2 = 1e-12

NVG = 8   # v chunks (8 batches each)
NVTAB = 8192   # per-core deduped t_emb shard rows (>= 64*101 padded uniques)
NUTAB = 4096   # per-core deduped c_emb shard rows

VA = (0, 1, 2, 3)
VB = (4, 5, 6, 7)

_CACHE = {}

_ACT_TABLE = "natural_log_exp_and_others"


def _patched_tables(orig_fn):
    def fn(arch):
        tabs = orig_fn(arch)
        return {
            name: (s if name == _ACT_TABLE else type(s)())
            for name, s in tabs.items()
        }
    return fn


_CFG = {
    "copy_mod": 3,       # 1-in-N copies go to DVE
    "cp_tsadd_act": 0,   # cp bias-copy on Act
    "trb": 3, "pdot": 2, "pout": 2,
    "stt_pool": 0,
    "sq_pool": 0,
    "trunc": 99,       # cos-scale on gpsimd
}


def _build_program():
    import os
    NOPATCH = os.environ.get("K_NOPATCH") == "1"
    import concourse.bass as bass
    import concourse.tile as tile
    import concourse.bacc as bacc_mod
    from concourse import bacc, mybir

    f32 = mybir.dt.float32
    bf16 = mybir.dt.bfloat16
    i32 = mybir.dt.int32
    i16 = mybir.dt.int16
    AF = mybir.ActivationFunctionType
    MUL = mybir.AluOpType.mult

    nc = bacc.Bacc(
        "TRN2",
        target_bir_lowering=False,
        debug=False,
        enable_asserts=False,
    )

    temb = nc.dram_tensor("temb", [V, E], bf16, kind="ExternalInput").ap()
    cemb = nc.dram_tensor("cemb", [V, E], bf16, kind="ExternalInput").ap()
    atw = nc.dram_tensor("atw", [128, 64], bf16, kind="ExternalInput").ap()
    acw = nc.dram_tensor("acw", [128, 64], bf16, kind="ExternalInput").ap()
    w2T = nc.dram_tensor("w2T", [128, 128], bf16, kind="ExternalInput").ap()
    atb = nc.dram_tensor("atb", [128, 1], f32, kind="ExternalInput").ap()
    acb = nc.dram_tensor("acb", [128, 1], f32, kind="ExternalInput").ap()
    eye = nc.dram_tensor("eye", [128, 128], bf16, kind="ExternalInput").ap()
    offt = nc.dram_tensor("offt", [128, 51], i32, kind="ExternalInput").ap()
    offc = nc.dram_tensor("offc", [128, BLOC // 2], i32, kind="ExternalInput").ap()
    maskT = nc.dram_tensor("maskT", [128, BLOC], f32, kind="ExternalInput").ap()
    out = nc.dram_tensor("out", [J, BLOC, E], bf16, kind="ExternalOutput").ap()

    with tile.TileContext(nc) as tc, ExitStack() as ctx:
        const = ctx.enter_context(tc.tile_pool(name="const", bufs=1))
        vgp = ctx.enter_context(tc.tile_pool(name="vg", bufs=13))
        ugp = ctx.enter_context(tc.tile_pool(name="ug", bufs=2))
        uTp = ctx.enter_context(tc.tile_pool(name="uT", bufs=8))
        vTp = ctx.enter_context(tc.tile_pool(name="vT", bufs=1))
        tpsp = ctx.enter_context(tc.tile_pool(name="tps", bufs=8))
        tp2p = ctx.enter_context(tc.tile_pool(name="tp2", bufs=8))
        cpsp = ctx.enter_context(tc.tile_pool(name="cps", bufs=4))
        cp2p = ctx.enter_context(tc.tile_pool(name="cp2", bufs=4))
        burp = ctx.enter_context(tc.tile_pool(name="bur", bufs=16))
        cosp = ctx.enter_context(tc.tile_pool(name="cos", bufs=3))
        attp = ctx.enter_context(tc.tile_pool(name="att", bufs=18))
        nrmp = ctx.enter_context(tc.tile_pool(name="nrm", bufs=1))
        osbp = ctx.enter_context(tc.tile_pool(name="osb", bufs=4))
        ptr = ctx.enter_context(tc.tile_pool(name="ptr", bufs=_CFG["trb"], space="PSUM"))
        pdot = ctx.enter_context(tc.tile_pool(name="pdot", bufs=_CFG["pdot"], space="PSUM"))
        pout = ctx.enter_context(tc.tile_pool(name="pout", bufs=_CFG["pout"], space="PSUM"))
        pvp = ctx.enter_context(tc.tile_pool(name="pv", bufs=1, space="PSUM"))

        # --- offset tiles first, so the gathers launch before other consts ---
        offc_t = const.tile([128, BLOC // 2], i32)
        nc.sync.dma_start(out=offc_t[:], in_=offc[:, :])
        offt_t = const.tile([128, 51], i32)
        nc.sync.dma_start(out=offt_t[:], in_=offt[:, :])

        # --- gathers: 1024 rows per indirect DMA ---
        ug = [None] * 4

        def u_gather(g):
            t = ugp.tile([128, 8, E], bf16, tag="ug")
            for j in range(8):
                nc.gpsimd.indirect_dma_start(
                    out=t[:, j, :], out_offset=None, in_=cemb[:, :],
                    in_offset=bass.IndirectOffsetOnAxis(
                        ap=offc_t[:, 8 * g + j : 8 * g + j + 1], axis=0))
            ug[g] = t


        vg = [None] * 13

        def v_gather(q):
            nblk = 3 if q == 12 else 4
            t = vgp.tile([128, 4, E], bf16, tag="vg")
            for j in range(nblk):
                nc.gpsimd.indirect_dma_start(
                    out=t[:, j, :], out_offset=None, in_=temb[:, :],
                    in_offset=bass.IndirectOffsetOnAxis(
                        ap=offt_t[:, 4 * q + j : 4 * q + j + 1], axis=0))
            vg[q] = t

        # interleave u among the v blocks: each cp chunk lands just before
        # the v chunks whose dots consume it; the final gather is one small
        # v group so only chunk 7 drains after it
        u_gather(0)
        for q in range(0, 5):
            v_gather(q)
        u_gather(1)
        for q in range(5, 8):
            v_gather(q)
        u_gather(2)
        for q in range(8, 10):
            v_gather(q)
        u_gather(3)
        for q in range(10, 13):
            v_gather(q)

        # --- remaining constants (overlap with the gathers) ---
        eye_t = const.tile([128, 128], bf16)
        nc.sync.dma_start(out=eye_t[:], in_=eye[:, :])
        atw_t = const.tile([128, 64], bf16)
        nc.sync.dma_start(out=atw_t[:], in_=atw[:, :])
        acw_t = const.tile([128, 64], bf16)
        nc.sync.dma_start(out=acw_t[:], in_=acw[:, :])
        w2T_t = const.tile([128, 128], bf16)
        nc.sync.dma_start(out=w2T_t[:], in_=w2T[:, :])
        atb_t = const.tile([128, 1], f32)
        nc.sync.dma_start(out=atb_t[:], in_=atb[:, :])
        acb_t = const.tile([128, 1], f32)
        nc.sync.dma_start(out=acb_t[:], in_=acb[:, :])
        maskT_t = const.tile([128, BLOC], f32)
        nc.sync.dma_start(out=maskT_t[:], in_=maskT[:, :])
        ones_bf = const.tile([128, 1], bf16)
        nc.vector.memset(ones_bf[:], 1.0)
        eps_t = const.tile([128, 1], f32)
        nc.vector.memset(eps_t[:], EPS2)

        # persistent PSUM vector bank: cols 0:64 nt2, 64:96 nc2, 96:160 rsum
        pvec = pvp.tile([128, 192], f32, tag="vec", space="PSUM")

        alt = [0]  # copy-engine alternator

        def copy_out(dst_ap, src_ap):
            if alt[0] % _CFG["copy_mod"] == 0:
                nc.vector.tensor_copy(out=dst_ap, in_=src_ap)
            else:
                nc.scalar.copy(out=dst_ap, in_=src_ap)
            alt[0] += 1

        # ---- u / cp / buR phase (per 16-batch chunk), software-pipelined ----
        uT = [None] * 8
        cps = [None] * 4
        cp2 = [None] * 4
        bur = [None] * 16

        def u_transp(c):
            for half in range(2):
                trb = ptr.tile([128, 512], bf16, tag="trb", space="PSUM")
                for q in range(4):
                    j = 4 * half + q
                    blk = 8 * c + j
                    nc.tensor.transpose(
                        out=trb[:, 128 * q : 128 * q + 128],
                        in_=ug[blk // 8][:, blk % 8, :], identity=eye_t[:, :])
                t = uTp.tile([128, 512], bf16, tag="uT")
                copy_out(t[:], trb[:, :])
                uT[2 * c + half] = t

        def u_work(c):
            # projections: 8 pairs -> 2 banks (rows 0:64 only), cps [64, 1024]
            cs = cpsp.tile([64, 1024], bf16, tag="cps")
            for h2 in range(2):
                pj = pdot.tile([128, 512], f32, tag="pd", space="PSUM")
                for s in range(4):
                    tt = 4 * h2 + s
                    pack = 2 * c + tt // 4
                    q = tt % 4
                    nc.tensor.matmul(
                        out=pj[0:64, 128 * s : 128 * s + 128],
                        lhsT=acw_t[:], rhs=uT[pack][:, 128 * q : 128 * q + 128],
                        start=True, stop=True)
                nc.vector.tensor_scalar_add(
                    cs[:, 512 * h2 : 512 * h2 + 512], pj[0:64, 0:512], acb_t[0:64, :])
            cps[c] = cs
            sq = cp2p.tile([64, 1024], bf16, tag="cp2")
            nc.vector.tensor_mul(out=sq[:], in0=cs[:], in1=cs[:])
            cp2[c] = sq
            # buR: one [64, 128] block per batch, 4 batches per bank
            for w in range(4):
                burb = pout.tile([128, 512], f32, tag="po", space="PSUM")
                for s3 in range(4):
                    bb16 = 4 * w + s3          # batch within chunk (0..15)
                    tt = bb16 // 2
                    eo = bb16 % 2
                    pack = 2 * c + tt // 4
                    q = tt % 4
                    nc.tensor.matmul(
                        out=burb[0:64, 128 * s3 : 128 * s3 + 128],
                        lhsT=uT[pack][:, 128 * q + 64 * eo : 128 * q + 64 * eo + 64],
                        rhs=w2T_t[:], start=True, stop=True)
                bt = burp.tile([64, 512], bf16, tag="bur")
                copy_out(bt[:], burb[0:64, :])
                bur[4 * c + w] = bt

        TR = _CFG["trunc"]
        TR = _CFG["trunc"]
        for c in range(4):
            if TR >= 2:
                u_transp(c)
            if c >= 1 and TR >= 3:
                u_work(c - 1)
        if TR >= 3:
            u_work(3)

        # nc2 norm matmuls (feed only the global Ln below): col = batch
        for c in range(4 if TR >= 3 else 0):
            sq = cp2[c]
            for bb16 in range(16):
                b = 16 * c + bb16
                nc.tensor.matmul(
                    out=pvec[0:64, 64 + b : 65 + b],
                    lhsT=sq[0:64, 64 * bb16 : 64 * bb16 + 64],
                    rhs=ones_bf[0:64, :], start=True, stop=True)

        if TR < 3:
            nc.compile if False else None
        lnc = nrmp.tile([64, 64], f32, tag="lnc")
        ncinv = nrmp.tile([64, 64], f32, tag="ncinv")
        if TR >= 3:
            nc.scalar.activation(out=lnc[:], in_=pvec[0:64, 64:128], func=AF.Ln,
                                 bias=eps_t[0:64, :])
            nc.scalar.activation(out=ncinv[:], in_=lnc[:], func=AF.Exp, scale=-0.5)

        # ---- v phase (per 8-batch chunk), transposes run one chunk ahead ----
        tps = [None] * 8
        vTs = [None] * 8
        tp2s = [None] * 8

        vTall = vTp.tile([128, 6528], bf16, tag="vTall")

        def v_transp(q):
            nblk = 3 if q == 12 else 4
            w = 128 * nblk
            trb = ptr.tile([128, 512], bf16, tag="trb", space="PSUM")
            for j in range(nblk):
                nc.tensor.transpose(
                    out=trb[:, 128 * j : 128 * j + 128],
                    in_=vg[q][:, j, :], identity=eye_t[:, :])
            copy_out(vTall[:, 512 * q : 512 * q + w], trb[:, 0:w])

        def v_proj(k):
            ts = tpsp.tile([64, 808], bf16, tag="tps")
            for h2 in range(2):
                projb = pdot.tile([128, 512], f32, tag="pd", space="PSUM")
                c0 = 808 * k + 404 * h2
                nc.tensor.matmul(out=projb[0:64, 0:404], lhsT=atw_t[:],
                                 rhs=vTall[:, c0 : c0 + 404], start=True, stop=True)
                nc.vector.tensor_scalar_add(
                    ts[:, 404 * h2 : 404 * h2 + 404], projb[0:64, 0:404], atb_t[0:64, :])
            tps[k] = ts
            sq = tp2p.tile([64, 808], bf16, tag="tp2")
            nc.vector.tensor_mul(out=sq[:], in0=ts[:], in1=ts[:])
            tp2s[k] = sq

        lnt = nrmp.tile([J, 64], f32, tag="lnt")
        ntinv = nrmp.tile([J, 64], f32, tag="ntinv")
        rsinv = nrmp.tile([J, 64], f32, tag="rsinv")

        def v_norms(kk):
            sq = tp2s[kk]
            for boff in range(8):
                b = 8 * kk + boff
                nc.tensor.matmul(
                    out=pvec[0:J, b : b + 1],
                    lhsT=sq[0:64, 101 * boff : 101 * boff + 101],
                    rhs=ones_bf[0:64, :], start=True, stop=True)

        def v_ntinv(kk):
            sl = slice(8 * kk, 8 * kk + 8)
            nc.scalar.activation(out=lnt[0:J, sl], in_=pvec[0:J, sl],
                                 func=AF.Ln, bias=eps_t[0:J, :])
            nc.scalar.activation(out=ntinv[0:J, sl], in_=lnt[0:J, sl],
                                 func=AF.Exp, scale=-0.5)

        # ---- attention stream (per 8-batch group), dots run one group ahead ----
        IA = {0: 0, 1: 1, 4: 2, 5: 3}
        IB = {2: 0, 3: 1, 6: 2, 7: 3}

        def attn_dots(g):
            dotb = pdot.tile([128, 512], f32, tag="pd", space="PSUM")
            for bb in range(_CFG.get("ndots", 8)):
                b = 8 * g + bb
                tt = (b % 16) // 2
                nc.tensor.matmul(
                    out=dotb[0:J, 64 * bb : 64 * bb + 64],
                    lhsT=tps[g][0:64, 101 * bb : 101 * bb + 101],
                    rhs=cps[b // 16][0:64,
                                     128 * tt + 64 * (b % 2) :
                                     128 * tt + 64 * (b % 2) + 64],
                    start=True, stop=True)
            cosg = cosp.tile([J, 512], bf16, tag="cos")
            if _CFG.get("no_stt"):
                nc.vector.memset(cosg[:], 0.0)
                return cosg
            nt_sl = ntinv[0:J, 8 * g : 8 * g + 8]
            in1 = bass.AP(tensor=nt_sl.tensor, offset=nt_sl.offset,
                          ap=[nt_sl.ap[0], nt_sl.ap[1], [0, 64]])
            eng = nc.gpsimd if _CFG["stt_pool"] else nc.vector
            if _CFG.get("plain_stt"):
                eng.tensor_tensor(out=cosg[:], in0=dotb[0:J, 0:512],
                                  in1=dotb[0:J, 0:512], op=MUL)
            else:
                eng.tensor_tensor(out=cosg[:], in0=dotb[0:J, 0:512], in1=in1, op=MUL)
            return cosg

        def attn_ab(g, cosg):
            ats = []
            for bb in range(8):
                ab = ptr.tile([128, 512], bf16, tag="trb", space="PSUM")
                nc.tensor.transpose(
                    out=ab[0:64, 0:J], in_=cosg[:, 64 * bb : 64 * bb + 64],
                    identity=eye_t[0:J, 0:J])
                ats.append(ab)
            for bb in range(8):
                b = 8 * g + bb
                at = attp.tile([64, J], bf16, tag="att")
                nc.scalar.activation(
                    out=at[:, :], in_=ats[bb][0:64, 0:J], func=AF.Exp,
                    scale=ncinv[:, b : b + 1],
                    bias=maskT_t[0:64, b : b + 1])
                ats[bb] = at
            return ats

        def attn_cd(g, ats):
            for bb in range(8):
                b = 8 * g + bb
                nc.tensor.matmul(
                    out=pvec[0:J, 128 + b : 129 + b],
                    lhsT=ats[bb][0:50, :], rhs=ones_bf[0:50, :],
                    start=True, stop=True)

            nc.vector.reciprocal(out=rsinv[0:J, 8 * g : 8 * g + 8],
                                 in_=pvec[0:J, 128 + 8 * g : 136 + 8 * g])

            for ob in range(2):
                outb = pout.tile([128, 512], f32, tag="po", space="PSUM")
                for bb4 in range(4):
                    b = 8 * g + 4 * ob + bb4
                    nc.tensor.matmul(
                        out=outb[0:J, 128 * bb4 : 128 * bb4 + 128],
                        lhsT=ats[4 * ob + bb4][0:50, :],
                        rhs=bur[b // 4][0:50, 128 * (b % 4) : 128 * (b % 4) + 128],
                        start=True, stop=True)
                osbt = osbp.tile([J, 4, E], bf16, tag="osb")
                ob_ap = outb[0:J, 0:512]
                in0 = bass.AP(tensor=ob_ap.tensor, offset=ob_ap.offset,
                              ap=[ob_ap.ap[0], [128, 4], [1, 128]])
                rs_sl = rsinv[0:J, 8 * g + 4 * ob : 8 * g + 4 * ob + 4]
                in1 = bass.AP(tensor=rs_sl.tensor, offset=rs_sl.offset,
                              ap=[rs_sl.ap[0], rs_sl.ap[1], [0, 128]])
                nc.vector.tensor_tensor(out=osbt[:], in0=in0, in1=in1, op=MUL)
                b0 = 8 * g + 4 * ob
                dst = bass.AP(tensor=out.tensor, offset=b0 * E,
                              ap=[[BLOC * E, J], [E, 4], [1, E]])
                nc.sync.dma_start(out=dst, in_=osbt[:])

        # unified loop: v chunks and attention stages interleaved so the
        # stream starts as soon as chunk 0 is projected; late v gathers are
        # emitted mid-loop so Pool can alternate gathers with cos-scales
        cos_store = {}
        ats_store = {}
        def full_chunk(kk):
            v_proj(kk)
            v_norms(kk)
            v_ntinv(kk)
            if TR >= 5:
                cosg = attn_dots(kk)
                if TR >= 6:
                    ats = attn_ab(kk, cosg)
                    if TR >= 7:
                        attn_cd(kk, ats)

        READY = {0: 1, 1: 3, 2: 4, 3: 6, 4: 7, 5: 9, 6: 11, 7: 12}
        next_k = [0]
        if TR >= 4:
            for q in range(13):
                v_transp(q)
                while next_k[0] < 8 and READY[next_k[0]] <= q:
                    full_chunk(next_k[0])
                    next_k[0] += 1

    if NOPATCH:
        nc.compile()
    else:
        orig = bacc_mod.get_activation_tables
        bacc_mod.get_activation_tables = _patched_tables(orig)
        try:
            nc.compile()
        finally:
            bacc_mod.get_activation_tables = orig
    return nc


def _get_program():
    if "nc" not in _CACHE:
        _CACHE["nc"] = _build_program()
    return _CACHE["nc"]


def _prep_inputs(batch_titems, batch_citems, batch_pad_ids, t_emb, c_emb,
                 Ac_w, Ac_b, At_w, At_b, Bc_w, Bc_b, R_w, R_b):
    import ml_dtypes
    bf = ml_dtypes.bfloat16
    f = lambda x: np.ascontiguousarray(np.asarray(x, dtype=np.float32))
    temb = np.ascontiguousarray(np.asarray(t_emb, np.float32).astype(bf))
    cemb = np.ascontiguousarray(np.asarray(c_emb, np.float32).astype(bf))
    tit = np.asarray(batch_titems).astype(np.int32)
    cit = np.asarray(batch_citems).astype(np.int32)
    pad = np.asarray(batch_pad_ids).astype(np.int64)

    mask = np.zeros((B, M), np.float32)
    mask[pad[0], pad[1]] = NEG

    At_w = f(At_w); Ac_w = f(Ac_w); Bc_w = f(Bc_w); R_w = f(R_w)
    At_b = f(At_b); Ac_b = f(Ac_b); Bc_b = f(Bc_b); R_b = f(R_b)

    atw = np.zeros((128, 64), np.float32)
    atw[:, 0:D] = At_w.T
    acw = np.zeros((128, 64), np.float32)
    acw[:, 0:D] = Ac_w.T
    w2T = np.ascontiguousarray((R_w @ Bc_w).T)

    atb = np.zeros((128, 1), np.float32)
    atb[0:D, 0] = At_b
    atb[64:64 + D, 0] = At_b
    acb = np.zeros((128, 1), np.float32)
    acb[0:D, 0] = Ac_b
    acb[64:64 + D, 0] = Ac_b

    rbeff = (R_b + R_w @ Bc_b).astype(np.float32)

    eye = np.eye(128, dtype=np.float32)

    atw_b = atw.astype(bf); acw_b = acw.astype(bf)
    w2T_b = w2T.astype(bf); eye_b = eye.astype(bf)

    in_maps = []
    for c in range(NCORES):
        s = c * BLOC
        # v offsets: tight-packed [128, 51]; col c = flat rows 128c..128c+127
        vflat = np.zeros(6528, np.int64)
        vflat[0:6464] = tit[s : s + BLOC].reshape(-1)
        offt = np.ascontiguousarray(vflat.reshape(51, 128).T).astype(np.int32)
        # u offsets: [128, 32], col t = pair (2t, 2t+1) at partition bases 0/64
        offc = np.zeros((128, BLOC // 2), np.int32)
        cslice = cit[s : s + BLOC]  # [64, 50]
        offc[0:M, :] = cslice[0::2].T
        offc[64:64 + M, :] = cslice[1::2].T
        # mask: [128, 64] col b rows 0:50 = mask[b], rows 50:64 = NEG (pads)
        mk = np.full((128, BLOC), NEG, np.float32)
        mk[0:M, :] = mask[s : s + BLOC].T

        in_maps.append({
            "temb": temb, "cemb": cemb,
            "atw": atw_b, "acw": acw_b, "w2T": w2T_b,
            "atb": atb, "acb": acb, "eye": eye_b,
            "offt": offt, "offc": offc, "maskT": mk,
        })
    return in_maps, rbeff


def run_sharded(in_maps, **kwargs):
    from concourse.bass_utils import run_bass_kernel_spmd

    nc = _get_program()
    res = run_bass_kernel_spmd(nc, in_maps, core_ids=list(range(NCORES)), **kwargs)
    outs = [np.asarray(res.results[c]["out"]).transpose(1, 0, 2).astype(np.float32)
            for c in range(NCORES)]
    full = np.concatenate(outs, axis=0)
    return full, res


def kernel(**inputs):
    in_maps, rbeff = _prep_inputs(**inputs)
    full, _ = run_sharded(in_maps)
    return (full + rbeff[None, None, :]).astype(np.float32)


# revision 47
# speedup vs baseline: 1.8443x; 1.0075x over previous
"""AttentiveItemToVec Trainium2 kernel (v2).

Full-input contract: kernel(**inputs) takes the unsharded numpy inputs and
returns the full [512, 101, 128] float32 output. Internally shards the batch
across 8 NeuronCores (64 batches each), runs a Bass/Tile kernel per core via
run_bass_kernel_spmd, and concatenates the per-core outputs.

v2 design (per core, 64 batches):
- Embedding tables converted to bf16 on host; 12 wide multi-row indirect
  DMAs (1024 rows each) amortize the ~1us SWDGE fixed cost per gather.
- All PE work in bf16 (1 cyc/row) with fp32 PSUM accumulation.
- v rows padded to 128/batch, u rows padded to 64/batch (2 batches per
  128-partition block) so every matmul operand sits at base partition 0/64.
- Projections computed feature-major in [*, 404/512]-wide banks, two
  D=60(+4 pad) blocks stacked at partitions 0/64 per bank; squared norms via
  one Act Square per bank + per-batch ones-matmuls into a persistent PSUM
  vector bank; 1/norm = exp(-0.5*ln(x+eps)) on two whole-bank Act ops.
- attn = exp applied m-major per batch pair ([0:50] and [64:114] rows of one
  bank; the [50:64] gap memset once per rotating buffer) with per-partition
  ncinv scale + additive -1e30 pad mask; softmax denominators via
  ones-matmul columns; normalization folded into the output stage.
- (R_w @ Bc_w) precomputed on host fuses the Bc/R projections into one
  matmul; Bc_b/R_b land in a host-side rbeff row added after the device run.
- Wide DVE ops use free-dim-stride-0 broadcast APs (ntinv over 50 cols,
  rsinv over 128 cols) so one op covers 8 resp. 4 batches.
"""

import numpy as np
from contextlib import ExitStack

V, E, D = 100000, 128, 60
B, J, M, P = 512, 101, 50, 5120
NCORES = 8
BLOC = B // NCORES  # 64
NEG = -1.0e30
EPS2 = 1e-12

NVG = 8   # v chunks (8 batches each)
NVTAB = 8192   # per-core deduped t_emb shard rows (>= 64*101 padded uniques)
NUTAB = 4096   # per-core deduped c_emb shard rows

VA = (0, 1, 2, 3)
VB = (4, 5, 6, 7)

_CACHE = {}

_ACT_TABLE = "natural_log_exp_and_others"


def _patched_tables(orig_fn):
    def fn(arch):
        tabs = orig_fn(arch)
        return {
            name: (s if name == _ACT_TABLE else type(s)())
            for name, s in tabs.items()
        }
    return fn


_CFG = {
    "copy_mod": 4,       # 1-in-N copies go to DVE
    "cp_tsadd_act": 0,   # cp bias-copy on Act
    "trb": 3, "pdot": 2, "pout": 2,
    "stt_pool": 0,
    "sq_pool": 0,
    "trunc": 99,       # cos-scale on gpsimd
}


def _build_program():
    import os
    NOPATCH = os.environ.get("K_NOPATCH") == "1"
    import concourse.bass as bass
    import concourse.tile as tile
    import concourse.bacc as bacc_mod
    from concourse import bacc, mybir

    f32 = mybir.dt.float32
    bf16 = mybir.dt.bfloat16
    i32 = mybir.dt.int32
    i16 = mybir.dt.int16
    AF = mybir.ActivationFunctionType
    MUL = mybir.AluOpType.mult

    nc = bacc.Bacc(
        "TRN2",
        target_bir_lowering=False,
        debug=False,
        enable_asserts=False,
    )

    temb = nc.dram_tensor("temb", [V, E], bf16, kind="ExternalInput").ap()
    cemb = nc.dram_tensor("cemb", [V, E], bf16, kind="ExternalInput").ap()
    atw = nc.dram_tensor("atw", [128, 64], bf16, kind="ExternalInput").ap()
    acw = nc.dram_tensor("acw", [128, 64], bf16, kind="ExternalInput").ap()
    w2T = nc.dram_tensor("w2T", [128, 128], bf16, kind="ExternalInput").ap()
    atb = nc.dram_tensor("atb", [128, 1], f32, kind="ExternalInput").ap()
    acb = nc.dram_tensor("acb", [128, 1], f32, kind="ExternalInput").ap()
    eye = nc.dram_tensor("eye", [128, 128], bf16, kind="ExternalInput").ap()
    offt = nc.dram_tensor("offt", [128, 51], i32, kind="ExternalInput").ap()
    offc = nc.dram_tensor("offc", [128, BLOC // 2], i32, kind="ExternalInput").ap()
    maskT = nc.dram_tensor("maskT", [128, BLOC], f32, kind="ExternalInput").ap()
    out = nc.dram_tensor("out", [J, BLOC, E], bf16, kind="ExternalOutput").ap()

    with tile.TileContext(nc) as tc, ExitStack() as ctx:
        const = ctx.enter_context(tc.tile_pool(name="const", bufs=1))
        vgp = ctx.enter_context(tc.tile_pool(name="vg", bufs=13))
        ugp = ctx.enter_context(tc.tile_pool(name="ug", bufs=2))
        uTp = ctx.enter_context(tc.tile_pool(name="uT", bufs=8))
        vTp = ctx.enter_context(tc.tile_pool(name="vT", bufs=1))
        tpsp = ctx.enter_context(tc.tile_pool(name="tps", bufs=8))
        tp2p = ctx.enter_context(tc.tile_pool(name="tp2", bufs=8))
        cpsp = ctx.enter_context(tc.tile_pool(name="cps", bufs=4))
        cp2p = ctx.enter_context(tc.tile_pool(name="cp2", bufs=4))
        burp = ctx.enter_context(tc.tile_pool(name="bur", bufs=16))
        cosp = ctx.enter_context(tc.tile_pool(name="cos", bufs=3))
        attp = ctx.enter_context(tc.tile_pool(name="att", bufs=18))
        nrmp = ctx.enter_context(tc.tile_pool(name="nrm", bufs=1))
        osbp = ctx.enter_context(tc.tile_pool(name="osb", bufs=4))
        ptr = ctx.enter_context(tc.tile_pool(name="ptr", bufs=_CFG["trb"], space="PSUM"))
        pdot = ctx.enter_context(tc.tile_pool(name="pdot", bufs=_CFG["pdot"], space="PSUM"))
        pout = ctx.enter_context(tc.tile_pool(name="pout", bufs=_CFG["pout"], space="PSUM"))
        pvp = ctx.enter_context(tc.tile_pool(name="pv", bufs=1, space="PSUM"))

        # --- offset tiles first, so the gathers launch before other consts ---
        offc_t = const.tile([128, BLOC // 2], i32)
        nc.sync.dma_start(out=offc_t[:], in_=offc[:, :])
        offt_t = const.tile([128, 51], i32)
        nc.sync.dma_start(out=offt_t[:], in_=offt[:, :])

        # --- gathers: 1024 rows per indirect DMA ---
        ug = [None] * 4

        def u_gather(g):
            t = ugp.tile([128, 8, E], bf16, tag="ug")
            for j in range(8):
                nc.gpsimd.indirect_dma_start(
                    out=t[:, j, :], out_offset=None, in_=cemb[:, :],
                    in_offset=bass.IndirectOffsetOnAxis(
                        ap=offc_t[:, 8 * g + j : 8 * g + j + 1], axis=0))
            ug[g] = t


        vg = [None] * 13

        def v_gather(q):
            nblk = 3 if q == 12 else 4
            t = vgp.tile([128, 4, E], bf16, tag="vg")
            for j in range(nblk):
                nc.gpsimd.indirect_dma_start(
                    out=t[:, j, :], out_offset=None, in_=temb[:, :],
                    in_offset=bass.IndirectOffsetOnAxis(
                        ap=offt_t[:, 4 * q + j : 4 * q + j + 1], axis=0))
            vg[q] = t

        # u first (cp chain feeds everything), then tight-packed v blocks
        for g in range(4):
            u_gather(g)
        for q in range(13):
            v_gather(q)

        # --- remaining constants (overlap with the gathers) ---
        eye_t = const.tile([128, 128], bf16)
        nc.sync.dma_start(out=eye_t[:], in_=eye[:, :])
        atw_t = const.tile([128, 64], bf16)
        nc.sync.dma_start(out=atw_t[:], in_=atw[:, :])
        acw_t = const.tile([128, 64], bf16)
        nc.sync.dma_start(out=acw_t[:], in_=acw[:, :])
        w2T_t = const.tile([128, 128], bf16)
        nc.sync.dma_start(out=w2T_t[:], in_=w2T[:, :])
        atb_t = const.tile([128, 1], f32)
        nc.sync.dma_start(out=atb_t[:], in_=atb[:, :])
        acb_t = const.tile([128, 1], f32)
        nc.sync.dma_start(out=acb_t[:], in_=acb[:, :])
        maskT_t = const.tile([128, BLOC], f32)
        nc.sync.dma_start(out=maskT_t[:], in_=maskT[:, :])
        ones_bf = const.tile([128, 1], bf16)
        nc.vector.memset(ones_bf[:], 1.0)
        eps_t = const.tile([128, 1], f32)
        nc.vector.memset(eps_t[:], EPS2)

        # persistent PSUM vector bank: cols 0:64 nt2, 64:96 nc2, 96:160 rsum
        pvec = pvp.tile([128, 192], f32, tag="vec", space="PSUM")

        alt = [0]  # copy-engine alternator

        def copy_out(dst_ap, src_ap):
            if alt[0] % _CFG["copy_mod"] == 0:
                nc.vector.tensor_copy(out=dst_ap, in_=src_ap)
            else:
                nc.scalar.copy(out=dst_ap, in_=src_ap)
            alt[0] += 1

        # ---- u / cp / buR phase (per 16-batch chunk), software-pipelined ----
        uT = [None] * 8
        cps = [None] * 4
        cp2 = [None] * 4
        bur = [None] * 16

        def u_transp(c):
            for half in range(2):
                trb = ptr.tile([128, 512], bf16, tag="trb", space="PSUM")
                for q in range(4):
                    j = 4 * half + q
                    blk = 8 * c + j
                    nc.tensor.transpose(
                        out=trb[:, 128 * q : 128 * q + 128],
                        in_=ug[blk // 8][:, blk % 8, :], identity=eye_t[:, :])
                t = uTp.tile([128, 512], bf16, tag="uT")
                copy_out(t[:], trb[:, :])
                uT[2 * c + half] = t

        def u_work(c):
            # projections: 8 pairs -> 2 banks (rows 0:64 only), cps [64, 1024]
            cs = cpsp.tile([64, 1024], bf16, tag="cps")
            for h2 in range(2):
                pj = pdot.tile([128, 512], f32, tag="pd", space="PSUM")
                for s in range(4):
                    tt = 4 * h2 + s
                    pack = 2 * c + tt // 4
                    q = tt % 4
                    nc.tensor.matmul(
                        out=pj[0:64, 128 * s : 128 * s + 128],
                        lhsT=acw_t[:], rhs=uT[pack][:, 128 * q : 128 * q + 128],
                        start=True, stop=True)
                nc.vector.tensor_scalar_add(
                    cs[:, 512 * h2 : 512 * h2 + 512], pj[0:64, 0:512], acb_t[0:64, :])
            cps[c] = cs
            sq = cp2p.tile([64, 1024], bf16, tag="cp2")
            nc.vector.tensor_mul(out=sq[:], in0=cs[:], in1=cs[:])
            cp2[c] = sq
            # buR: one [64, 128] block per batch, 4 batches per bank
            for w in range(4):
                burb = pout.tile([128, 512], f32, tag="po", space="PSUM")
                for s3 in range(4):
                    bb16 = 4 * w + s3          # batch within chunk (0..15)
                    tt = bb16 // 2
                    eo = bb16 % 2
                    pack = 2 * c + tt // 4
                    q = tt % 4
                    nc.tensor.matmul(
                        out=burb[0:64, 128 * s3 : 128 * s3 + 128],
                        lhsT=uT[pack][:, 128 * q + 64 * eo : 128 * q + 64 * eo + 64],
                        rhs=w2T_t[:], start=True, stop=True)
                bt = burp.tile([64, 512], bf16, tag="bur")
                copy_out(bt[:], burb[0:64, :])
                bur[4 * c + w] = bt

        TR = _CFG["trunc"]
        TR = _CFG["trunc"]
        for c in range(4):
            if TR >= 2:
                u_transp(c)
            if c >= 1 and TR >= 3:
                u_work(c - 1)
        if TR >= 3:
            u_work(3)

        # nc2 norm matmuls (feed only the global Ln below): col = batch
        for c in range(4 if TR >= 3 else 0):
            sq = cp2[c]
            for bb16 in range(16):
                b = 16 * c + bb16
                nc.tensor.matmul(
                    out=pvec[0:64, 64 + b : 65 + b],
                    lhsT=sq[0:64, 64 * bb16 : 64 * bb16 + 64],
                    rhs=ones_bf[0:64, :], start=True, stop=True)

        if TR < 3:
            nc.compile if False else None
        lnc = nrmp.tile([64, 64], f32, tag="lnc")
        ncinv = nrmp.tile([64, 64], f32, tag="ncinv")
        if TR >= 3:
            nc.scalar.activation(out=lnc[:], in_=pvec[0:64, 64:128], func=AF.Ln,
                                 bias=eps_t[0:64, :])
            nc.scalar.activation(out=ncinv[:], in_=lnc[:], func=AF.Exp, scale=-0.5)

        # ---- v phase (per 8-batch chunk), transposes run one chunk ahead ----
        tps = [None] * 8
        vTs = [None] * 8
        tp2s = [None] * 8

        vTall = vTp.tile([128, 6528], bf16, tag="vTall")

        def v_transp(q):
            nblk = 3 if q == 12 else 4
            w = 128 * nblk
            trb = ptr.tile([128, 512], bf16, tag="trb", space="PSUM")
            for j in range(nblk):
                nc.tensor.transpose(
                    out=trb[:, 128 * j : 128 * j + 128],
                    in_=vg[q][:, j, :], identity=eye_t[:, :])
            copy_out(vTall[:, 512 * q : 512 * q + w], trb[:, 0:w])

        def v_proj(k):
            ts = tpsp.tile([64, 808], bf16, tag="tps")
            for h2 in range(2):
                projb = pdot.tile([128, 512], f32, tag="pd", space="PSUM")
                c0 = 808 * k + 404 * h2
                nc.tensor.matmul(out=projb[0:64, 0:404], lhsT=atw_t[:],
                                 rhs=vTall[:, c0 : c0 + 404], start=True, stop=True)
                nc.vector.tensor_scalar_add(
                    ts[:, 404 * h2 : 404 * h2 + 404], projb[0:64, 0:404], atb_t[0:64, :])
            tps[k] = ts
            sq = tp2p.tile([64, 808], bf16, tag="tp2")
            nc.vector.tensor_mul(out=sq[:], in0=ts[:], in1=ts[:])
            tp2s[k] = sq

        lnt = nrmp.tile([J, 64], f32, tag="lnt")
        ntinv = nrmp.tile([J, 64], f32, tag="ntinv")
        rsinv = nrmp.tile([J, 64], f32, tag="rsinv")

        def v_norms(kk):
            sq = tp2s[kk]
            for boff in range(8):
                b = 8 * kk + boff
                nc.tensor.matmul(
                    out=pvec[0:J, b : b + 1],
                    lhsT=sq[0:64, 101 * boff : 101 * boff + 101],
                    rhs=ones_bf[0:64, :], start=True, stop=True)

        def v_ntinv(kk):
            sl = slice(8 * kk, 8 * kk + 8)
            nc.scalar.activation(out=lnt[0:J, sl], in_=pvec[0:J, sl],
                                 func=AF.Ln, bias=eps_t[0:J, :])
            nc.scalar.activation(out=ntinv[0:J, sl], in_=lnt[0:J, sl],
                                 func=AF.Exp, scale=-0.5)

        # ---- attention stream (per 8-batch group), dots run one group ahead ----
        IA = {0: 0, 1: 1, 4: 2, 5: 3}
        IB = {2: 0, 3: 1, 6: 2, 7: 3}

        def attn_dots(g):
            dotb = pdot.tile([128, 512], f32, tag="pd", space="PSUM")
            for bb in range(_CFG.get("ndots", 8)):
                b = 8 * g + bb
                tt = (b % 16) // 2
                nc.tensor.matmul(
                    out=dotb[0:J, 64 * bb : 64 * bb + 64],
                    lhsT=tps[g][0:64, 101 * bb : 101 * bb + 101],
                    rhs=cps[b // 16][0:64,
                                     128 * tt + 64 * (b % 2) :
                                     128 * tt + 64 * (b % 2) + 64],
                    start=True, stop=True)
            cosg = cosp.tile([J, 512], bf16, tag="cos")
            if _CFG.get("no_stt"):
                nc.vector.memset(cosg[:], 0.0)
                return cosg
            nt_sl = ntinv[0:J, 8 * g : 8 * g + 8]
            in1 = bass.AP(tensor=nt_sl.tensor, offset=nt_sl.offset,
                          ap=[nt_sl.ap[0], nt_sl.ap[1], [0, 64]])
            eng = nc.gpsimd if _CFG["stt_pool"] else nc.vector
            if _CFG.get("plain_stt"):
                eng.tensor_tensor(out=cosg[:], in0=dotb[0:J, 0:512],
                                  in1=dotb[0:J, 0:512], op=MUL)
            else:
                eng.tensor_tensor(out=cosg[:], in0=dotb[0:J, 0:512], in1=in1, op=MUL)
            return cosg

        def attn_ab(g, cosg):
            ats = []
            for bb in range(8):
                ab = ptr.tile([128, 512], bf16, tag="trb", space="PSUM")
                nc.tensor.transpose(
                    out=ab[0:64, 0:J], in_=cosg[:, 64 * bb : 64 * bb + 64],
                    identity=eye_t[0:J, 0:J])
                ats.append(ab)
            for bb in range(8):
                b = 8 * g + bb
                at = attp.tile([64, J], bf16, tag="att")
                nc.scalar.activation(
                    out=at[:, :], in_=ats[bb][0:64, 0:J], func=AF.Exp,
                    scale=ncinv[:, b : b + 1],
                    bias=maskT_t[0:64, b : b + 1])
                ats[bb] = at
            return ats

        def attn_cd(g, ats):
            for bb in range(8):
                b = 8 * g + bb
                nc.tensor.matmul(
                    out=pvec[0:J, 128 + b : 129 + b],
                    lhsT=ats[bb][0:50, :], rhs=ones_bf[0:50, :],
                    start=True, stop=True)

            nc.vector.reciprocal(out=rsinv[0:J, 8 * g : 8 * g + 8],
                                 in_=pvec[0:J, 128 + 8 * g : 136 + 8 * g])

            for ob in range(2):
                outb = pout.tile([128, 512], f32, tag="po", space="PSUM")
                for bb4 in range(4):
                    b = 8 * g + 4 * ob + bb4
                    nc.tensor.matmul(
                        out=outb[0:J, 128 * bb4 : 128 * bb4 + 128],
                        lhsT=ats[4 * ob + bb4][0:50, :],
                        rhs=bur[b // 4][0:50, 128 * (b % 4) : 128 * (b % 4) + 128],
                        start=True, stop=True)
                osbt = osbp.tile([J, 4, E], bf16, tag="osb")
                ob_ap = outb[0:J, 0:512]
                in0 = bass.AP(tensor=ob_ap.tensor, offset=ob_ap.offset,
                              ap=[ob_ap.ap[0], [128, 4], [1, 128]])
                rs_sl = rsinv[0:J, 8 * g + 4 * ob : 8 * g + 4 * ob + 4]
                in1 = bass.AP(tensor=rs_sl.tensor, offset=rs_sl.offset,
                              ap=[rs_sl.ap[0], rs_sl.ap[1], [0, 128]])
                nc.vector.tensor_tensor(out=osbt[:], in0=in0, in1=in1, op=MUL)
                b0 = 8 * g + 4 * ob
                dst = bass.AP(tensor=out.tensor, offset=b0 * E,
                              ap=[[BLOC * E, J], [E, 4], [1, E]])
                nc.sync.dma_start(out=dst, in_=osbt[:])

        # unified loop: v chunks and attention stages interleaved so the
        # stream starts as soon as chunk 0 is projected; late v gathers are
        # emitted mid-loop so Pool can alternate gathers with cos-scales
        cos_store = {}
        ats_store = {}
        def full_chunk(kk):
            v_proj(kk)
            v_norms(kk)
            v_ntinv(kk)
            if TR >= 5:
                cosg = attn_dots(kk)
                if TR >= 6:
                    ats = attn_ab(kk, cosg)
                    if TR >= 7:
                        attn_cd(kk, ats)

        READY = {0: 1, 1: 3, 2: 4, 3: 6, 4: 7, 5: 9, 6: 11, 7: 12}
        next_k = [0]
        if TR >= 4:
            for q in range(13):
                v_transp(q)
                while next_k[0] < 8 and READY[next_k[0]] <= q:
                    full_chunk(next_k[0])
                    next_k[0] += 1

    if NOPATCH:
        nc.compile()
    else:
        orig = bacc_mod.get_activation_tables
        bacc_mod.get_activation_tables = _patched_tables(orig)
        try:
            nc.compile()
        finally:
            bacc_mod.get_activation_tables = orig
    return nc


def _get_program():
    if "nc" not in _CACHE:
        _CACHE["nc"] = _build_program()
    return _CACHE["nc"]


def _prep_inputs(batch_titems, batch_citems, batch_pad_ids, t_emb, c_emb,
                 Ac_w, Ac_b, At_w, At_b, Bc_w, Bc_b, R_w, R_b):
    import ml_dtypes
    bf = ml_dtypes.bfloat16
    f = lambda x: np.ascontiguousarray(np.asarray(x, dtype=np.float32))
    temb = np.ascontiguousarray(np.asarray(t_emb, np.float32).astype(bf))
    cemb = np.ascontiguousarray(np.asarray(c_emb, np.float32).astype(bf))
    tit = np.asarray(batch_titems).astype(np.int32)
    cit = np.asarray(batch_citems).astype(np.int32)
    pad = np.asarray(batch_pad_ids).astype(np.int64)

    mask = np.zeros((B, M), np.float32)
    mask[pad[0], pad[1]] = NEG

    At_w = f(At_w); Ac_w = f(Ac_w); Bc_w = f(Bc_w); R_w = f(R_w)
    At_b = f(At_b); Ac_b = f(Ac_b); Bc_b = f(Bc_b); R_b = f(R_b)

    atw = np.zeros((128, 64), np.float32)
    atw[:, 0:D] = At_w.T
    acw = np.zeros((128, 64), np.float32)
    acw[:, 0:D] = Ac_w.T
    w2T = np.ascontiguousarray((R_w @ Bc_w).T)

    atb = np.zeros((128, 1), np.float32)
    atb[0:D, 0] = At_b
    atb[64:64 + D, 0] = At_b
    acb = np.zeros((128, 1), np.float32)
    acb[0:D, 0] = Ac_b
    acb[64:64 + D, 0] = Ac_b

    rbeff = (R_b + R_w @ Bc_b).astype(np.float32)

    eye = np.eye(128, dtype=np.float32)

    atw_b = atw.astype(bf); acw_b = acw.astype(bf)
    w2T_b = w2T.astype(bf); eye_b = eye.astype(bf)

    in_maps = []
    for c in range(NCORES):
        s = c * BLOC
        # v offsets: tight-packed [128, 51]; col c = flat rows 128c..128c+127
        vflat = np.zeros(6528, np.int64)
        vflat[0:6464] = tit[s : s + BLOC].reshape(-1)
        offt = np.ascontiguousarray(vflat.reshape(51, 128).T).astype(np.int32)
        # u offsets: [128, 32], col t = pair (2t, 2t+1) at partition bases 0/64
        offc = np.zeros((128, BLOC // 2), np.int32)
        cslice = cit[s : s + BLOC]  # [64, 50]
        offc[0:M, :] = cslice[0::2].T
        offc[64:64 + M, :] = cslice[1::2].T
        # mask: [128, 64] col b rows 0:50 = mask[b], rows 50:64 = NEG (pads)
        mk = np.full((128, BLOC), NEG, np.float32)
        mk[0:M, :] = mask[s : s + BLOC].T

        in_maps.append({
            "temb": temb, "cemb": cemb,
            "atw": atw_b, "acw": acw_b, "w2T": w2T_b,
            "atb": atb, "acb": acb, "eye": eye_b,
            "offt": offt, "offc": offc, "maskT": mk,
        })
    return in_maps, rbeff


def run_sharded(in_maps, **kwargs):
    from concourse.bass_utils import run_bass_kernel_spmd

    nc = _get_program()
    res = run_bass_kernel_spmd(nc, in_maps, core_ids=list(range(NCORES)), **kwargs)
    outs = [np.asarray(res.results[c]["out"]).transpose(1, 0, 2).astype(np.float32)
            for c in range(NCORES)]
    full = np.concatenate(outs, axis=0)
    return full, res


def kernel(**inputs):
    in_maps, rbeff = _prep_inputs(**inputs)
    full, _ = run_sharded(in_maps)
    return (full + rbeff[None, None, :]).astype(np.float32)


# revision 51
# speedup vs baseline: 1.9417x; 1.0528x over previous
"""AttentiveItemToVec Trainium2 kernel (v2).

Full-input contract: kernel(**inputs) takes the unsharded numpy inputs and
returns the full [512, 101, 128] float32 output. Internally shards the batch
across 8 NeuronCores (64 batches each), runs a Bass/Tile kernel per core via
run_bass_kernel_spmd, and concatenates the per-core outputs.

v2 design (per core, 64 batches):
- Embedding tables converted to bf16 on host; 12 wide multi-row indirect
  DMAs (1024 rows each) amortize the ~1us SWDGE fixed cost per gather.
- All PE work in bf16 (1 cyc/row) with fp32 PSUM accumulation.
- v rows padded to 128/batch, u rows padded to 64/batch (2 batches per
  128-partition block) so every matmul operand sits at base partition 0/64.
- Projections computed feature-major in [*, 404/512]-wide banks, two
  D=60(+4 pad) blocks stacked at partitions 0/64 per bank; squared norms via
  one Act Square per bank + per-batch ones-matmuls into a persistent PSUM
  vector bank; 1/norm = exp(-0.5*ln(x+eps)) on two whole-bank Act ops.
- attn = exp applied m-major per batch pair ([0:50] and [64:114] rows of one
  bank; the [50:64] gap memset once per rotating buffer) with per-partition
  ncinv scale + additive -1e30 pad mask; softmax denominators via
  ones-matmul columns; normalization folded into the output stage.
- (R_w @ Bc_w) precomputed on host fuses the Bc/R projections into one
  matmul; Bc_b/R_b land in a host-side rbeff row added after the device run.
- Wide DVE ops use free-dim-stride-0 broadcast APs (ntinv over 50 cols,
  rsinv over 128 cols) so one op covers 8 resp. 4 batches.
"""

import numpy as np
from contextlib import ExitStack

V, E, D = 100000, 128, 60
B, J, M, P = 512, 101, 50, 5120
NCORES = 8
BLOC = B // NCORES  # 64
NEG = -1.0e30
EPS2 = 1e-12

NVG = 8   # v chunks (8 batches each)
NVTAB = 8192   # per-core deduped t_emb shard rows (>= 64*101 padded uniques)
NUTAB = 4096   # per-core deduped c_emb shard rows

VA = (0, 1, 2, 3)
VB = (4, 5, 6, 7)

_CACHE = {}

_ACT_TABLE = "natural_log_exp_and_others"


def _patched_tables(orig_fn):
    def fn(arch):
        tabs = orig_fn(arch)
        return {
            name: (s if name == _ACT_TABLE else type(s)())
            for name, s in tabs.items()
        }
    return fn


_CFG = {
    "copy_mod": 4,       # 1-in-N copies go to DVE
    "cp_tsadd_act": 0,   # cp bias-copy on Act
    "trb": 3, "pdot": 2, "pout": 2,
    "stt_pool": 0,
    "sq_pool": 0,
    "trunc": 99,       # cos-scale on gpsimd
}


def _build_program():
    import os
    NOPATCH = os.environ.get("K_NOPATCH") == "1"
    import concourse.bass as bass
    import concourse.tile as tile
    import concourse.bacc as bacc_mod
    from concourse import bacc, mybir

    f32 = mybir.dt.float32
    bf16 = mybir.dt.bfloat16
    i32 = mybir.dt.int32
    i16 = mybir.dt.int16
    AF = mybir.ActivationFunctionType
    MUL = mybir.AluOpType.mult

    nc = bacc.Bacc(
        "TRN2",
        target_bir_lowering=False,
        debug=False,
        enable_asserts=False,
    )

    temb = nc.dram_tensor("temb", [V, E], bf16, kind="ExternalInput").ap()
    cemb = nc.dram_tensor("cemb", [V, E], bf16, kind="ExternalInput").ap()
    atw = nc.dram_tensor("atw", [128, 64], bf16, kind="ExternalInput").ap()
    acw = nc.dram_tensor("acw", [128, 64], bf16, kind="ExternalInput").ap()
    w2T = nc.dram_tensor("w2T", [128, 128], bf16, kind="ExternalInput").ap()
    atb = nc.dram_tensor("atb", [128, 1], f32, kind="ExternalInput").ap()
    acb = nc.dram_tensor("acb", [128, 1], f32, kind="ExternalInput").ap()
    eye = nc.dram_tensor("eye", [128, 128], bf16, kind="ExternalInput").ap()
    offt = nc.dram_tensor("offt", [128, 51], i32, kind="ExternalInput").ap()
    offc = nc.dram_tensor("offc", [128, BLOC // 2], i32, kind="ExternalInput").ap()
    maskT = nc.dram_tensor("maskT", [128, BLOC], f32, kind="ExternalInput").ap()
    out = nc.dram_tensor("out", [J, BLOC, E], bf16, kind="ExternalOutput").ap()

    with tile.TileContext(nc) as tc, ExitStack() as ctx:
        const = ctx.enter_context(tc.tile_pool(name="const", bufs=1))
        vgp = ctx.enter_context(tc.tile_pool(name="vg", bufs=13))
        ugp = ctx.enter_context(tc.tile_pool(name="ug", bufs=2))
        uTp = ctx.enter_context(tc.tile_pool(name="uT", bufs=8))
        vTp = ctx.enter_context(tc.tile_pool(name="vT", bufs=1))
        tpsp = ctx.enter_context(tc.tile_pool(name="tps", bufs=8))
        tp2p = ctx.enter_context(tc.tile_pool(name="tp2", bufs=8))
        cpsp = ctx.enter_context(tc.tile_pool(name="cps", bufs=4))
        cp2p = ctx.enter_context(tc.tile_pool(name="cp2", bufs=4))
        burp = ctx.enter_context(tc.tile_pool(name="bur", bufs=16))
        cosp = ctx.enter_context(tc.tile_pool(name="cos", bufs=3))
        attp = ctx.enter_context(tc.tile_pool(name="att", bufs=18))
        nrmp = ctx.enter_context(tc.tile_pool(name="nrm", bufs=1))
        osbp = ctx.enter_context(tc.tile_pool(name="osb", bufs=4))
        ptr = ctx.enter_context(tc.tile_pool(name="ptr", bufs=_CFG["trb"], space="PSUM"))
        pdot = ctx.enter_context(tc.tile_pool(name="pdot", bufs=_CFG["pdot"], space="PSUM"))
        pout = ctx.enter_context(tc.tile_pool(name="pout", bufs=_CFG["pout"], space="PSUM"))
        pvp = ctx.enter_context(tc.tile_pool(name="pv", bufs=1, space="PSUM"))

        # --- offset tiles first, so the gathers launch before other consts ---
        offc_t = const.tile([128, BLOC // 2], i32)
        nc.sync.dma_start(out=offc_t[:], in_=offc[:, :])
        offt_t = const.tile([128, 51], i32)
        nc.sync.dma_start(out=offt_t[:], in_=offt[:, :])

        # --- gathers: 1024 rows per indirect DMA ---
        ug = [None] * 4

        def u_gather(g):
            t = ugp.tile([128, 8, E], bf16, tag="ug")
            for j in range(8):
                nc.gpsimd.indirect_dma_start(
                    out=t[:, j, :], out_offset=None, in_=cemb[:, :],
                    in_offset=bass.IndirectOffsetOnAxis(
                        ap=offc_t[:, 8 * g + j : 8 * g + j + 1], axis=0))
            ug[g] = t


        vg = [None] * 13

        def v_gather(q):
            nblk = 3 if q == 12 else 4
            t = vgp.tile([128, 4, E], bf16, tag="vg")
            for j in range(nblk):
                nc.gpsimd.indirect_dma_start(
                    out=t[:, j, :], out_offset=None, in_=temb[:, :],
                    in_offset=bass.IndirectOffsetOnAxis(
                        ap=offt_t[:, 4 * q + j : 4 * q + j + 1], axis=0))
            vg[q] = t

        # u first (cp chain feeds everything), then tight-packed v blocks
        for g in range(4):
            u_gather(g)
        for q in range(13):
            v_gather(q)

        # --- remaining constants (overlap with the gathers) ---
        eye_t = const.tile([128, 128], bf16)
        nc.sync.dma_start(out=eye_t[:], in_=eye[:, :])
        atw_t = const.tile([128, 64], bf16)
        nc.sync.dma_start(out=atw_t[:], in_=atw[:, :])
        acw_t = const.tile([128, 64], bf16)
        nc.sync.dma_start(out=acw_t[:], in_=acw[:, :])
        w2T_t = const.tile([128, 128], bf16)
        nc.sync.dma_start(out=w2T_t[:], in_=w2T[:, :])
        atb_t = const.tile([128, 1], f32)
        nc.sync.dma_start(out=atb_t[:], in_=atb[:, :])
        acb_t = const.tile([128, 1], f32)
        nc.sync.dma_start(out=acb_t[:], in_=acb[:, :])
        maskT_t = const.tile([128, BLOC], f32)
        nc.sync.dma_start(out=maskT_t[:], in_=maskT[:, :])
        ones_bf = const.tile([128, 1], bf16)
        nc.vector.memset(ones_bf[:], 1.0)
        eps_t = const.tile([128, 1], f32)
        nc.vector.memset(eps_t[:], EPS2)

        # persistent PSUM vector bank: cols 0:64 nt2, 64:96 nc2, 96:160 rsum
        pvec = pvp.tile([128, 192], f32, tag="vec", space="PSUM")

        alt = [0]  # copy-engine alternator

        def copy_out(dst_ap, src_ap):
            if alt[0] % _CFG["copy_mod"] == 0:
                nc.vector.tensor_copy(out=dst_ap, in_=src_ap)
            else:
                nc.scalar.copy(out=dst_ap, in_=src_ap)
            alt[0] += 1

        # ---- u / cp / buR phase (per 16-batch chunk), software-pipelined ----
        uT = [None] * 8
        cps = [None] * 4
        cp2 = [None] * 4
        bur = [None] * 16

        def u_transp(c):
            for half in range(2):
                trb = ptr.tile([128, 512], bf16, tag="trb", space="PSUM")
                for q in range(4):
                    j = 4 * half + q
                    blk = 8 * c + j
                    nc.tensor.transpose(
                        out=trb[:, 128 * q : 128 * q + 128],
                        in_=ug[blk // 8][:, blk % 8, :], identity=eye_t[:, :])
                t = uTp.tile([128, 512], bf16, tag="uT")
                copy_out(t[:], trb[:, :])
                uT[2 * c + half] = t

        def u_work(c):
            # projections: 8 pairs -> 2 banks (rows 0:64 only), cps [64, 1024]
            cs = cpsp.tile([64, 1024], bf16, tag="cps")
            sq = cp2p.tile([64, 1024], bf16, tag="cp2")
            for h2 in range(2):
                pj = pdot.tile([128, 512], f32, tag="pd", space="PSUM")
                for s in range(4):
                    tt = 4 * h2 + s
                    pack = 2 * c + tt // 4
                    q = tt % 4
                    nc.tensor.matmul(
                        out=pj[0:64, 128 * s : 128 * s + 128],
                        lhsT=acw_t[:], rhs=uT[pack][:, 128 * q : 128 * q + 128],
                        start=True, stop=True)
                nc.vector.tensor_scalar_add(
                    cs[:, 512 * h2 : 512 * h2 + 512], pj[0:64, 0:512], acb_t[0:64, :])
                nc.scalar.activation(
                    out=sq[:, 512 * h2 : 512 * h2 + 512], in_=pj[0:64, 0:512],
                    func=AF.Square, bias=acb_t[0:64, :], scale=1.0)
            cps[c] = cs
            cp2[c] = sq
            # buR: one [64, 128] block per batch, 4 batches per bank
            for w in range(4):
                burb = pout.tile([128, 512], f32, tag="po", space="PSUM")
                for s3 in range(4):
                    bb16 = 4 * w + s3          # batch within chunk (0..15)
                    tt = bb16 // 2
                    eo = bb16 % 2
                    pack = 2 * c + tt // 4
                    q = tt % 4
                    nc.tensor.matmul(
                        out=burb[0:64, 128 * s3 : 128 * s3 + 128],
                        lhsT=uT[pack][:, 128 * q + 64 * eo : 128 * q + 64 * eo + 64],
                        rhs=w2T_t[:], start=True, stop=True)
                bt = burp.tile([64, 512], bf16, tag="bur")
                copy_out(bt[:], burb[0:64, :])
                bur[4 * c + w] = bt

        TR = _CFG["trunc"]
        TR = _CFG["trunc"]
        for c in range(4):
            if TR >= 2:
                u_transp(c)
            if c >= 1 and TR >= 3:
                u_work(c - 1)
        if TR >= 3:
            u_work(3)

        # nc2 norm matmuls (feed only the global Ln below): col = batch
        for c in range(4 if TR >= 3 else 0):
            sq = cp2[c]
            for bb16 in range(16):
                b = 16 * c + bb16
                nc.tensor.matmul(
                    out=pvec[0:64, 64 + b : 65 + b],
                    lhsT=sq[0:64, 64 * bb16 : 64 * bb16 + 64],
                    rhs=ones_bf[0:64, :], start=True, stop=True)

        if TR < 3:
            nc.compile if False else None
        lnc = nrmp.tile([64, 64], f32, tag="lnc")
        ncinv = nrmp.tile([64, 64], f32, tag="ncinv")
        if TR >= 3:
            nc.scalar.activation(out=lnc[:], in_=pvec[0:64, 64:128], func=AF.Ln,
                                 bias=eps_t[0:64, :])
            nc.scalar.activation(out=ncinv[:], in_=lnc[:], func=AF.Exp, scale=-0.5)

        # ---- v phase (per 8-batch chunk), transposes run one chunk ahead ----
        tps = [None] * 8
        vTs = [None] * 8
        tp2s = [None] * 8

        vTall = vTp.tile([128, 6528], bf16, tag="vTall")

        def v_transp(q):
            nblk = 3 if q == 12 else 4
            w = 128 * nblk
            trb = ptr.tile([128, 512], bf16, tag="trb", space="PSUM")
            for j in range(nblk):
                nc.tensor.transpose(
                    out=trb[:, 128 * j : 128 * j + 128],
                    in_=vg[q][:, j, :], identity=eye_t[:, :])
            copy_out(vTall[:, 512 * q : 512 * q + w], trb[:, 0:w])

        def v_proj_h(k, h2):
            if h2 == 0:
                ts_new = tpsp.tile([64, 808], bf16, tag="tps")
                sq_new = tp2p.tile([64, 808], bf16, tag="tp2")
                tps[k] = ts_new
                tp2s[k] = sq_new
            ts, sq = tps[k], tp2s[k]
            projb = pdot.tile([128, 512], f32, tag="pd", space="PSUM")
            c0 = 808 * k + 404 * h2
            nc.tensor.matmul(out=projb[0:64, 0:404], lhsT=atw_t[:],
                             rhs=vTall[:, c0 : c0 + 404], start=True, stop=True)
            nc.vector.tensor_scalar_add(
                ts[:, 404 * h2 : 404 * h2 + 404], projb[0:64, 0:404], atb_t[0:64, :])
            nc.scalar.activation(
                out=sq[:, 404 * h2 : 404 * h2 + 404], in_=projb[0:64, 0:404],
                func=AF.Square, bias=atb_t[0:64, :], scale=1.0)

        lnt = nrmp.tile([J, 64], f32, tag="lnt")
        ntinv = nrmp.tile([J, 64], f32, tag="ntinv")
        rsinv = nrmp.tile([J, 64], f32, tag="rsinv")

        def v_norms_h(kk, h2):
            sq = tp2s[kk]
            for boff in range(4 * h2, 4 * h2 + 4):
                b = 8 * kk + boff
                nc.tensor.matmul(
                    out=pvec[0:J, b : b + 1],
                    lhsT=sq[0:64, 101 * boff : 101 * boff + 101],
                    rhs=ones_bf[0:64, :], start=True, stop=True)

        def v_ntinv_h(kk, h2):
            sl = slice(8 * kk + 4 * h2, 8 * kk + 4 * h2 + 4)
            nc.scalar.activation(out=lnt[0:J, sl], in_=pvec[0:J, sl],
                                 func=AF.Ln, bias=eps_t[0:J, :])
            nc.scalar.activation(out=ntinv[0:J, sl], in_=lnt[0:J, sl],
                                 func=AF.Exp, scale=-0.5)

        # ---- attention stream (per 8-batch group), dots run one group ahead ----
        IA = {0: 0, 1: 1, 4: 2, 5: 3}
        IB = {2: 0, 3: 1, 6: 2, 7: 3}

        dot_store = {}

        def attn_dots_h(g, h2):
            if h2 == 0:
                dotb_new = pdot.tile([128, 512], f32, tag="pd", space="PSUM")
                cosg_new = cosp.tile([J, 512], bf16, tag="cos")
                dot_store[g] = (dotb_new, cosg_new)
            dotb, cosg = dot_store[g]
            for bb in range(4 * h2, 4 * h2 + 4):
                b = 8 * g + bb
                tt = (b % 16) // 2
                nc.tensor.matmul(
                    out=dotb[0:J, 64 * bb : 64 * bb + 64],
                    lhsT=tps[g][0:64, 101 * bb : 101 * bb + 101],
                    rhs=cps[b // 16][0:64,
                                     128 * tt + 64 * (b % 2) :
                                     128 * tt + 64 * (b % 2) + 64],
                    start=True, stop=True)
            nt_sl = ntinv[0:J, 8 * g + 4 * h2 : 8 * g + 4 * h2 + 4]
            in1 = bass.AP(tensor=nt_sl.tensor, offset=nt_sl.offset,
                          ap=[nt_sl.ap[0], nt_sl.ap[1], [0, 64]])
            nc.vector.tensor_tensor(
                out=cosg[:, 256 * h2 : 256 * h2 + 256],
                in0=dotb[0:J, 256 * h2 : 256 * h2 + 256], in1=in1, op=MUL)
            return cosg

        def attn_ab_h(g, h2, cosg):
            ats = []
            for bb in range(4 * h2, 4 * h2 + 4):
                ab = ptr.tile([128, 512], bf16, tag="trb", space="PSUM")
                nc.tensor.transpose(
                    out=ab[0:64, 0:J], in_=cosg[:, 64 * bb : 64 * bb + 64],
                    identity=eye_t[0:J, 0:J])
                ats.append(ab)
            for i, bb in enumerate(range(4 * h2, 4 * h2 + 4)):
                b = 8 * g + bb
                at = attp.tile([64, J], bf16, tag="att")
                nc.scalar.activation(
                    out=at[:, :], in_=ats[i][0:64, 0:J], func=AF.Exp,
                    scale=ncinv[:, b : b + 1],
                    bias=maskT_t[0:64, b : b + 1])
                ats[i] = at
            return ats

        def attn_cd_h(g, h2, ats):
            for i, bb in enumerate(range(4 * h2, 4 * h2 + 4)):
                b = 8 * g + bb
                nc.tensor.matmul(
                    out=pvec[0:J, 128 + b : 129 + b],
                    lhsT=ats[i][0:50, :], rhs=ones_bf[0:50, :],
                    start=True, stop=True)

            b0 = 8 * g + 4 * h2
            nc.vector.reciprocal(out=rsinv[0:J, b0 : b0 + 4],
                                 in_=pvec[0:J, 128 + b0 : 132 + b0])

            outb = pout.tile([128, 512], f32, tag="po", space="PSUM")
            for bb4 in range(4):
                b = b0 + bb4
                nc.tensor.matmul(
                    out=outb[0:J, 128 * bb4 : 128 * bb4 + 128],
                    lhsT=ats[bb4][0:50, :],
                    rhs=bur[b // 4][0:50, 128 * (b % 4) : 128 * (b % 4) + 128],
                    start=True, stop=True)
            osbt = osbp.tile([J, 4, E], bf16, tag="osb")
            if h2 == 1:
                for bb4 in range(4):
                    b = b0 + bb4
                    nc.scalar.activation(
                        out=osbt[0:J, bb4, :],
                        in_=outb[0:J, 128 * bb4 : 128 * bb4 + 128],
                        func=AF.Copy, scale=rsinv[0:J, b : b + 1])
            else:
                ob_ap = outb[0:J, 0:512]
                in0 = bass.AP(tensor=ob_ap.tensor, offset=ob_ap.offset,
                              ap=[ob_ap.ap[0], [128, 4], [1, 128]])
                rs_sl = rsinv[0:J, b0 : b0 + 4]
                in1 = bass.AP(tensor=rs_sl.tensor, offset=rs_sl.offset,
                              ap=[rs_sl.ap[0], rs_sl.ap[1], [0, 128]])
                nc.vector.tensor_tensor(out=osbt[:], in0=in0, in1=in1, op=MUL)
            dst = bass.AP(tensor=out.tensor, offset=b0 * E,
                          ap=[[BLOC * E, J], [E, 4], [1, E]])
            nc.sync.dma_start(out=dst, in_=osbt[:])

        # unified loop: v chunks and attention stages interleaved so the
        # stream starts as soon as chunk 0 is projected; late v gathers are
        # emitted mid-loop so Pool can alternate gathers with cos-scales
        cos_store = {}
        ats_store = {}
        def full_chunk(kk):
            for h2 in range(2):
                v_proj_h(kk, h2)
            for h2 in range(2):
                v_norms_h(kk, h2)
                v_ntinv_h(kk, h2)
            if TR >= 5:
                for h2 in range(2):
                    cosg = attn_dots_h(kk, h2)
                if TR >= 6:
                    for h2 in range(2):
                        ats = attn_ab_h(kk, h2, cosg)
                        if TR >= 7:
                            attn_cd_h(kk, h2, ats)

        READY = {0: 1, 1: 3, 2: 4, 3: 6, 4: 7, 5: 9, 6: 11, 7: 12}
        next_k = [0]
        if TR >= 4:
            for q in range(13):
                v_transp(q)
                while next_k[0] < 8 and READY[next_k[0]] <= q:
                    full_chunk(next_k[0])
                    next_k[0] += 1

    if NOPATCH:
        nc.compile()
    else:
        orig = bacc_mod.get_activation_tables
        bacc_mod.get_activation_tables = _patched_tables(orig)
        try:
            nc.compile()
        finally:
            bacc_mod.get_activation_tables = orig
    return nc


def _get_program():
    if "nc" not in _CACHE:
        _CACHE["nc"] = _build_program()
    return _CACHE["nc"]


def _prep_inputs(batch_titems, batch_citems, batch_pad_ids, t_emb, c_emb,
                 Ac_w, Ac_b, At_w, At_b, Bc_w, Bc_b, R_w, R_b):
    import ml_dtypes
    bf = ml_dtypes.bfloat16
    f = lambda x: np.ascontiguousarray(np.asarray(x, dtype=np.float32))
    temb = np.ascontiguousarray(np.asarray(t_emb, np.float32).astype(bf))
    cemb = np.ascontiguousarray(np.asarray(c_emb, np.float32).astype(bf))
    tit = np.asarray(batch_titems).astype(np.int32)
    cit = np.asarray(batch_citems).astype(np.int32)
    pad = np.asarray(batch_pad_ids).astype(np.int64)

    mask = np.zeros((B, M), np.float32)
    mask[pad[0], pad[1]] = NEG

    At_w = f(At_w); Ac_w = f(Ac_w); Bc_w = f(Bc_w); R_w = f(R_w)
    At_b = f(At_b); Ac_b = f(Ac_b); Bc_b = f(Bc_b); R_b = f(R_b)

    atw = np.zeros((128, 64), np.float32)
    atw[:, 0:D] = At_w.T
    acw = np.zeros((128, 64), np.float32)
    acw[:, 0:D] = Ac_w.T
    w2T = np.ascontiguousarray((R_w @ Bc_w).T)

    atb = np.zeros((128, 1), np.float32)
    atb[0:D, 0] = At_b
    atb[64:64 + D, 0] = At_b
    acb = np.zeros((128, 1), np.float32)
    acb[0:D, 0] = Ac_b
    acb[64:64 + D, 0] = Ac_b

    rbeff = (R_b + R_w @ Bc_b).astype(np.float32)

    eye = np.eye(128, dtype=np.float32)

    atw_b = atw.astype(bf); acw_b = acw.astype(bf)
    w2T_b = w2T.astype(bf); eye_b = eye.astype(bf)

    in_maps = []
    for c in range(NCORES):
        s = c * BLOC
        # v offsets: tight-packed [128, 51]; col c = flat rows 128c..128c+127
        vflat = np.zeros(6528, np.int64)
        vflat[0:6464] = tit[s : s + BLOC].reshape(-1)
        offt = np.ascontiguousarray(vflat.reshape(51, 128).T).astype(np.int32)
        # u offsets: [128, 32], col t = pair (2t, 2t+1) at partition bases 0/64
        offc = np.zeros((128, BLOC // 2), np.int32)
        cslice = cit[s : s + BLOC]  # [64, 50]
        offc[0:M, :] = cslice[0::2].T
        offc[64:64 + M, :] = cslice[1::2].T
        # mask: [128, 64] col b rows 0:50 = mask[b], rows 50:64 = NEG (pads)
        mk = np.full((128, BLOC), NEG, np.float32)
        mk[0:M, :] = mask[s : s + BLOC].T

        in_maps.append({
            "temb": temb, "cemb": cemb,
            "atw": atw_b, "acw": acw_b, "w2T": w2T_b,
            "atb": atb, "acb": acb, "eye": eye_b,
            "offt": offt, "offc": offc, "maskT": mk,
        })
    return in_maps, rbeff


def run_sharded(in_maps, **kwargs):
    from concourse.bass_utils import run_bass_kernel_spmd

    nc = _get_program()
    res = run_bass_kernel_spmd(nc, in_maps, core_ids=list(range(NCORES)), **kwargs)
    outs = [np.asarray(res.results[c]["out"]).transpose(1, 0, 2).astype(np.float32)
            for c in range(NCORES)]
    full = np.concatenate(outs, axis=0)
    return full, res


def kernel(**inputs):
    in_maps, rbeff = _prep_inputs(**inputs)
    full, _ = run_sharded(in_maps)
    return (full + rbeff[None, None, :]).astype(np.float32)


# revision 52
# speedup vs baseline: 2.0930x; 1.0779x over previous
"""AttentiveItemToVec Trainium2 kernel (v2).

Full-input contract: kernel(**inputs) takes the unsharded numpy inputs and
returns the full [512, 101, 128] float32 output. Internally shards the batch
across 8 NeuronCores (64 batches each), runs a Bass/Tile kernel per core via
run_bass_kernel_spmd, and concatenates the per-core outputs.

v2 design (per core, 64 batches):
- Embedding tables converted to bf16 on host; 12 wide multi-row indirect
  DMAs (1024 rows each) amortize the ~1us SWDGE fixed cost per gather.
- All PE work in bf16 (1 cyc/row) with fp32 PSUM accumulation.
- v rows padded to 128/batch, u rows padded to 64/batch (2 batches per
  128-partition block) so every matmul operand sits at base partition 0/64.
- Projections computed feature-major in [*, 404/512]-wide banks, two
  D=60(+4 pad) blocks stacked at partitions 0/64 per bank; squared norms via
  one Act Square per bank + per-batch ones-matmuls into a persistent PSUM
  vector bank; 1/norm = exp(-0.5*ln(x+eps)) on two whole-bank Act ops.
- attn = exp applied m-major per batch pair ([0:50] and [64:114] rows of one
  bank; the [50:64] gap memset once per rotating buffer) with per-partition
  ncinv scale + additive -1e30 pad mask; softmax denominators via
  ones-matmul columns; normalization folded into the output stage.
- (R_w @ Bc_w) precomputed on host fuses the Bc/R projections into one
  matmul; Bc_b/R_b land in a host-side rbeff row added after the device run.
- Wide DVE ops use free-dim-stride-0 broadcast APs (ntinv over 50 cols,
  rsinv over 128 cols) so one op covers 8 resp. 4 batches.
"""

import numpy as np
from contextlib import ExitStack

V, E, D = 100000, 128, 60
B, J, M, P = 512, 101, 50, 5120
NCORES = 8
BLOC = B // NCORES  # 64
NEG = -1.0e30
EPS2 = 1e-12

NVG = 8   # v chunks (8 batches each)
NVTAB = 8192   # per-core deduped t_emb shard rows (>= 64*101 padded uniques)
NUTAB = 4096   # per-core deduped c_emb shard rows

VA = (0, 1, 2, 3)
VB = (4, 5, 6, 7)

_CACHE = {}

_ACT_TABLE = "natural_log_exp_and_others"


def _patched_tables(orig_fn):
    def fn(arch):
        tabs = orig_fn(arch)
        return {
            name: (s if name == _ACT_TABLE else type(s)())
            for name, s in tabs.items()
        }
    return fn


_CFG = {
    "copy_mod": 4,       # 1-in-N copies go to DVE
    "cp_tsadd_act": 0,   # cp bias-copy on Act
    "trb": 3, "pdot": 2, "pout": 2,
    "stt_pool": 0,
    "sq_pool": 0,
    "trunc": 99,       # cos-scale on gpsimd
}


def _build_program():
    import os
    NOPATCH = os.environ.get("K_NOPATCH") == "1"
    import concourse.bass as bass
    import concourse.tile as tile
    import concourse.bacc as bacc_mod
    from concourse import bacc, mybir

    f32 = mybir.dt.float32
    bf16 = mybir.dt.bfloat16
    i32 = mybir.dt.int32
    i16 = mybir.dt.int16
    AF = mybir.ActivationFunctionType
    MUL = mybir.AluOpType.mult

    nc = bacc.Bacc(
        "TRN2",
        target_bir_lowering=False,
        debug=False,
        enable_asserts=False,
    )

    temb = nc.dram_tensor("temb", [V, E], bf16, kind="ExternalInput").ap()
    cemb = nc.dram_tensor("cemb", [V, E], bf16, kind="ExternalInput").ap()
    atw = nc.dram_tensor("atw", [128, 64], bf16, kind="ExternalInput").ap()
    acw = nc.dram_tensor("acw", [128, 64], bf16, kind="ExternalInput").ap()
    w2T = nc.dram_tensor("w2T", [128, 128], bf16, kind="ExternalInput").ap()
    atb = nc.dram_tensor("atb", [128, 1], f32, kind="ExternalInput").ap()
    acb = nc.dram_tensor("acb", [128, 1], f32, kind="ExternalInput").ap()
    eye = nc.dram_tensor("eye", [128, 128], bf16, kind="ExternalInput").ap()
    offt = nc.dram_tensor("offt", [128, 51], i32, kind="ExternalInput").ap()
    offc = nc.dram_tensor("offc", [128, 25], i32, kind="ExternalInput").ap()
    maskT = nc.dram_tensor("maskT", [128, BLOC], f32, kind="ExternalInput").ap()
    out = nc.dram_tensor("out", [J, BLOC, E], bf16, kind="ExternalOutput").ap()

    with tile.TileContext(nc) as tc, ExitStack() as ctx:
        const = ctx.enter_context(tc.tile_pool(name="const", bufs=1))
        vgp = ctx.enter_context(tc.tile_pool(name="vg", bufs=13))
        ugp = ctx.enter_context(tc.tile_pool(name="ug", bufs=2))
        uTp = ctx.enter_context(tc.tile_pool(name="uT", bufs=1))
        vTp = ctx.enter_context(tc.tile_pool(name="vT", bufs=1))
        tpsp = ctx.enter_context(tc.tile_pool(name="tps", bufs=8))
        tp2p = ctx.enter_context(tc.tile_pool(name="tp2", bufs=8))
        cpsp = ctx.enter_context(tc.tile_pool(name="cps", bufs=4))
        cp2p = ctx.enter_context(tc.tile_pool(name="cp2", bufs=4))
        burp = ctx.enter_context(tc.tile_pool(name="bur", bufs=16))
        cosp = ctx.enter_context(tc.tile_pool(name="cos", bufs=3))
        attp = ctx.enter_context(tc.tile_pool(name="att", bufs=18))
        nrmp = ctx.enter_context(tc.tile_pool(name="nrm", bufs=1))
        osbp = ctx.enter_context(tc.tile_pool(name="osb", bufs=4))
        ptr = ctx.enter_context(tc.tile_pool(name="ptr", bufs=_CFG["trb"], space="PSUM"))
        pdot = ctx.enter_context(tc.tile_pool(name="pdot", bufs=_CFG["pdot"], space="PSUM"))
        pout = ctx.enter_context(tc.tile_pool(name="pout", bufs=_CFG["pout"], space="PSUM"))
        pvp = ctx.enter_context(tc.tile_pool(name="pv", bufs=1, space="PSUM"))

        # --- offset tiles first, so the gathers launch before other consts ---
        offc_t = const.tile([128, 25], i32)
        nc.sync.dma_start(out=offc_t[:], in_=offc[:, :])
        offt_t = const.tile([128, 51], i32)
        nc.sync.dma_start(out=offt_t[:], in_=offt[:, :])

        # --- gathers: 1024 rows per indirect DMA ---
        ug = [None] * 7

        def u_gather(g):
            nblk = 1 if g == 6 else 4
            t = ugp.tile([128, 4, E], bf16, tag="ug")
            for j in range(nblk):
                nc.gpsimd.indirect_dma_start(
                    out=t[:, j, :], out_offset=None, in_=cemb[:, :],
                    in_offset=bass.IndirectOffsetOnAxis(
                        ap=offc_t[:, 4 * g + j : 4 * g + j + 1], axis=0))
            ug[g] = t


        vg = [None] * 13

        def v_gather(q):
            nblk = 3 if q == 12 else 4
            t = vgp.tile([128, 4, E], bf16, tag="vg")
            for j in range(nblk):
                nc.gpsimd.indirect_dma_start(
                    out=t[:, j, :], out_offset=None, in_=temb[:, :],
                    in_offset=bass.IndirectOffsetOnAxis(
                        ap=offt_t[:, 4 * q + j : 4 * q + j + 1], axis=0))
            vg[q] = t

        # u first (cp chain feeds everything), then tight-packed v blocks
        for g in range(7):
            u_gather(g)
        for q in range(13):
            v_gather(q)

        # --- remaining constants (overlap with the gathers) ---
        eye_t = const.tile([128, 128], bf16)
        nc.sync.dma_start(out=eye_t[:], in_=eye[:, :])
        atw_t = const.tile([128, 64], bf16)
        nc.sync.dma_start(out=atw_t[:], in_=atw[:, :])
        acw_t = const.tile([128, 64], bf16)
        nc.sync.dma_start(out=acw_t[:], in_=acw[:, :])
        w2T_t = const.tile([128, 128], bf16)
        nc.sync.dma_start(out=w2T_t[:], in_=w2T[:, :])
        atb_t = const.tile([128, 1], f32)
        nc.sync.dma_start(out=atb_t[:], in_=atb[:, :])
        acb_t = const.tile([128, 1], f32)
        nc.sync.dma_start(out=acb_t[:], in_=acb[:, :])
        maskT_t = const.tile([128, BLOC], f32)
        nc.sync.dma_start(out=maskT_t[:], in_=maskT[:, :])
        ones_bf = const.tile([128, 1], bf16)
        nc.vector.memset(ones_bf[:], 1.0)
        eps_t = const.tile([128, 1], f32)
        nc.vector.memset(eps_t[:], EPS2)

        # persistent PSUM vector bank: cols 0:64 nt2, 64:96 nc2, 96:160 rsum
        pvec = pvp.tile([128, 192], f32, tag="vec", space="PSUM")

        alt = [0]  # copy-engine alternator

        def copy_out(dst_ap, src_ap):
            if alt[0] % _CFG["copy_mod"] == 0:
                nc.vector.tensor_copy(out=dst_ap, in_=src_ap)
            else:
                nc.scalar.copy(out=dst_ap, in_=src_ap)
            alt[0] += 1

        # ---- u / cp / buR phase (per 16-batch chunk), software-pipelined ----
        cps = [None] * 4
        cp2 = [None] * 4
        bur = [None] * 16

        uTall = uTp.tile([128, 3216], bf16, tag="uTall")
        nc.vector.memset(uTall[:, 3200:3216], 0.0)

        def u_transp(q2):
            nblk = 1 if q2 == 6 else 4
            w = 128 * nblk
            trb = ptr.tile([128, 512], bf16, tag="trb", space="PSUM")
            for j in range(nblk):
                nc.tensor.transpose(
                    out=trb[:, 128 * j : 128 * j + 128],
                    in_=ug[q2][:, j, :], identity=eye_t[:, :])
            copy_out(uTall[:, 512 * q2 : 512 * q2 + w], trb[:, 0:w])

        def u_work(c):
            # 8 pair-projections [64, 100] + a 14-col tail pad, cps [64, 816]
            cs = cpsp.tile([64, 816], bf16, tag="cps")
            sq = cp2p.tile([64, 816], bf16, tag="cp2")
            for h2 in range(2):
                pj = pdot.tile([128, 512], f32, tag="pd", space="PSUM")
                wcols = 414 if h2 == 1 else 400
                for s in range(4):
                    tt = 4 * h2 + s
                    c0 = 100 * (8 * c + tt)
                    wc = 114 if tt == 7 else 100
                    nc.tensor.matmul(
                        out=pj[0:64, 100 * s : 100 * s + wc],
                        lhsT=acw_t[:], rhs=uTall[:, c0 : c0 + wc],
                        start=True, stop=True)
                nc.vector.tensor_scalar_add(
                    cs[:, 400 * h2 : 400 * h2 + wcols], pj[0:64, 0:wcols], acb_t[0:64, :])
                nc.scalar.activation(
                    out=sq[:, 400 * h2 : 400 * h2 + wcols], in_=pj[0:64, 0:wcols],
                    func=AF.Square, bias=acb_t[0:64, :], scale=1.0)
            cps[c] = cs
            cp2[c] = sq
            # buR: one [64, 128] block per batch, 4 batches per bank
            for w in range(4):
                burb = pout.tile([128, 512], f32, tag="po", space="PSUM")
                for s3 in range(4):
                    bb16 = 4 * w + s3          # batch within chunk (0..15)
                    b = 16 * c + bb16
                    nc.tensor.matmul(
                        out=burb[0:64, 128 * s3 : 128 * s3 + 128],
                        lhsT=uTall[:, 50 * b : 50 * b + 64],
                        rhs=w2T_t[:], start=True, stop=True)
                bt = burp.tile([64, 512], bf16, tag="bur")
                copy_out(bt[:], burb[0:64, :])
                bur[4 * c + w] = bt

        TR = _CFG["trunc"]
        TR = _CFG["trunc"]
        TR = _CFG["trunc"]
        UREADY = {0: 2, 1: 4, 2: 5, 3: 7}
        un = [0]
        for q2 in range(7):
            if TR >= 2:
                u_transp(q2)
            while un[0] < 4 and UREADY[un[0]] <= q2 + 1 and TR >= 3:
                u_work(un[0])
                un[0] += 1

        # nc2 norm matmuls (feed only the global Ln below): col = batch
        for c in range(4 if TR >= 3 else 0):
            sq = cp2[c]
            for bb16 in range(16):
                b = 16 * c + bb16
                nc.tensor.matmul(
                    out=pvec[0:64, 64 + b : 65 + b],
                    lhsT=sq[0:64, 50 * bb16 : 50 * bb16 + 64],
                    rhs=ones_bf[0:64, :], start=True, stop=True)

        if TR < 3:
            nc.compile if False else None
        lnc = nrmp.tile([64, 64], f32, tag="lnc")
        ncinv = nrmp.tile([64, 64], f32, tag="ncinv")
        if TR >= 3:
            nc.scalar.activation(out=lnc[:], in_=pvec[0:64, 64:128], func=AF.Ln,
                                 bias=eps_t[0:64, :])
            nc.scalar.activation(out=ncinv[:], in_=lnc[:], func=AF.Exp, scale=-0.5)

        # ---- v phase (per 8-batch chunk), transposes run one chunk ahead ----
        tps = [None] * 8
        vTs = [None] * 8
        tp2s = [None] * 8

        vTall = vTp.tile([128, 6528], bf16, tag="vTall")

        def v_transp(q):
            nblk = 3 if q == 12 else 4
            w = 128 * nblk
            trb = ptr.tile([128, 512], bf16, tag="trb", space="PSUM")
            for j in range(nblk):
                nc.tensor.transpose(
                    out=trb[:, 128 * j : 128 * j + 128],
                    in_=vg[q][:, j, :], identity=eye_t[:, :])
            copy_out(vTall[:, 512 * q : 512 * q + w], trb[:, 0:w])

        def v_proj_h(k, h2):
            if h2 == 0:
                ts_new = tpsp.tile([64, 808], bf16, tag="tps")
                sq_new = tp2p.tile([64, 808], bf16, tag="tp2")
                tps[k] = ts_new
                tp2s[k] = sq_new
            ts, sq = tps[k], tp2s[k]
            projb = pdot.tile([128, 512], f32, tag="pd", space="PSUM")
            c0 = 808 * k + 404 * h2
            nc.tensor.matmul(out=projb[0:64, 0:404], lhsT=atw_t[:],
                             rhs=vTall[:, c0 : c0 + 404], start=True, stop=True)
            nc.vector.tensor_scalar_add(
                ts[:, 404 * h2 : 404 * h2 + 404], projb[0:64, 0:404], atb_t[0:64, :])
            nc.scalar.activation(
                out=sq[:, 404 * h2 : 404 * h2 + 404], in_=projb[0:64, 0:404],
                func=AF.Square, bias=atb_t[0:64, :], scale=1.0)

        lnt = nrmp.tile([J, 64], f32, tag="lnt")
        ntinv = nrmp.tile([J, 64], f32, tag="ntinv")
        rsinv = nrmp.tile([J, 64], f32, tag="rsinv")

        def v_norms_h(kk, h2):
            sq = tp2s[kk]
            for boff in range(4 * h2, 4 * h2 + 4):
                b = 8 * kk + boff
                nc.tensor.matmul(
                    out=pvec[0:J, b : b + 1],
                    lhsT=sq[0:64, 101 * boff : 101 * boff + 101],
                    rhs=ones_bf[0:64, :], start=True, stop=True)

        def v_ntinv_h(kk, h2):
            sl = slice(8 * kk + 4 * h2, 8 * kk + 4 * h2 + 4)
            nc.scalar.activation(out=lnt[0:J, sl], in_=pvec[0:J, sl],
                                 func=AF.Ln, bias=eps_t[0:J, :])
            nc.scalar.activation(out=ntinv[0:J, sl], in_=lnt[0:J, sl],
                                 func=AF.Exp, scale=-0.5)

        # ---- attention stream (per 8-batch group), dots run one group ahead ----
        IA = {0: 0, 1: 1, 4: 2, 5: 3}
        IB = {2: 0, 3: 1, 6: 2, 7: 3}

        dot_store = {}

        def attn_dots_h(g, h2):
            if h2 == 0:
                dotb_new = pdot.tile([128, 512], f32, tag="pd", space="PSUM")
                cosg_new = cosp.tile([J, 512], bf16, tag="cos")
                dot_store[g] = (dotb_new, cosg_new)
            dotb, cosg = dot_store[g]
            for bb in range(4 * h2, 4 * h2 + 4):
                b = 8 * g + bb
                tt = (b % 16) // 2
                nc.tensor.matmul(
                    out=dotb[0:J, 64 * bb : 64 * bb + 64],
                    lhsT=tps[g][0:64, 101 * bb : 101 * bb + 101],
                    rhs=cps[b // 16][0:64,
                                     100 * tt + 50 * (b % 2) :
                                     100 * tt + 50 * (b % 2) + 64],
                    start=True, stop=True)
            nt_sl = ntinv[0:J, 8 * g + 4 * h2 : 8 * g + 4 * h2 + 4]
            in1 = bass.AP(tensor=nt_sl.tensor, offset=nt_sl.offset,
                          ap=[nt_sl.ap[0], nt_sl.ap[1], [0, 64]])
            nc.vector.tensor_tensor(
                out=cosg[:, 256 * h2 : 256 * h2 + 256],
                in0=dotb[0:J, 256 * h2 : 256 * h2 + 256], in1=in1, op=MUL)
            return cosg

        def attn_ab_h(g, h2, cosg):
            ats = []
            for bb in range(4 * h2, 4 * h2 + 4):
                ab = ptr.tile([128, 512], bf16, tag="trb", space="PSUM")
                nc.tensor.transpose(
                    out=ab[0:64, 0:J], in_=cosg[:, 64 * bb : 64 * bb + 64],
                    identity=eye_t[0:J, 0:J])
                ats.append(ab)
            for i, bb in enumerate(range(4 * h2, 4 * h2 + 4)):
                b = 8 * g + bb
                at = attp.tile([64, J], bf16, tag="att")
                nc.scalar.activation(
                    out=at[:, :], in_=ats[i][0:64, 0:J], func=AF.Exp,
                    scale=ncinv[:, b : b + 1],
                    bias=maskT_t[0:64, b : b + 1])
                ats[i] = at
            return ats

        def attn_cd_h(g, h2, ats):
            for i, bb in enumerate(range(4 * h2, 4 * h2 + 4)):
                b = 8 * g + bb
                nc.tensor.matmul(
                    out=pvec[0:J, 128 + b : 129 + b],
                    lhsT=ats[i][0:50, :], rhs=ones_bf[0:50, :],
                    start=True, stop=True)

            b0 = 8 * g + 4 * h2
            nc.vector.reciprocal(out=rsinv[0:J, b0 : b0 + 4],
                                 in_=pvec[0:J, 128 + b0 : 132 + b0])

            outb = pout.tile([128, 512], f32, tag="po", space="PSUM")
            for bb4 in range(4):
                b = b0 + bb4
                nc.tensor.matmul(
                    out=outb[0:J, 128 * bb4 : 128 * bb4 + 128],
                    lhsT=ats[bb4][0:50, :],
                    rhs=bur[b // 4][0:50, 128 * (b % 4) : 128 * (b % 4) + 128],
                    start=True, stop=True)
            osbt = osbp.tile([J, 4, E], bf16, tag="osb")
            if h2 == 1:
                for bb4 in range(4):
                    b = b0 + bb4
                    nc.scalar.activation(
                        out=osbt[0:J, bb4, :],
                        in_=outb[0:J, 128 * bb4 : 128 * bb4 + 128],
                        func=AF.Copy, scale=rsinv[0:J, b : b + 1])
            else:
                ob_ap = outb[0:J, 0:512]
                in0 = bass.AP(tensor=ob_ap.tensor, offset=ob_ap.offset,
                              ap=[ob_ap.ap[0], [128, 4], [1, 128]])
                rs_sl = rsinv[0:J, b0 : b0 + 4]
                in1 = bass.AP(tensor=rs_sl.tensor, offset=rs_sl.offset,
                              ap=[rs_sl.ap[0], rs_sl.ap[1], [0, 128]])
                nc.vector.tensor_tensor(out=osbt[:], in0=in0, in1=in1, op=MUL)
            dst = bass.AP(tensor=out.tensor, offset=b0 * E,
                          ap=[[BLOC * E, J], [E, 4], [1, E]])
            nc.sync.dma_start(out=dst, in_=osbt[:])

        # unified loop: v chunks and attention stages interleaved so the
        # stream starts as soon as chunk 0 is projected; late v gathers are
        # emitted mid-loop so Pool can alternate gathers with cos-scales
        cos_store = {}
        ats_store = {}
        def full_chunk(kk):
            for h2 in range(2):
                v_proj_h(kk, h2)
            for h2 in range(2):
                v_norms_h(kk, h2)
                v_ntinv_h(kk, h2)
            if TR >= 5:
                for h2 in range(2):
                    cosg = attn_dots_h(kk, h2)
                if TR >= 6:
                    for h2 in range(2):
                        ats = attn_ab_h(kk, h2, cosg)
                        if TR >= 7:
                            attn_cd_h(kk, h2, ats)

        READY = {0: 1, 1: 3, 2: 4, 3: 6, 4: 7, 5: 9, 6: 11, 7: 12}
        next_k = [0]
        if TR >= 4:
            for q in range(13):
                v_transp(q)
                while next_k[0] < 8 and READY[next_k[0]] <= q:
                    full_chunk(next_k[0])
                    next_k[0] += 1

    if NOPATCH:
        nc.compile()
    else:
        orig = bacc_mod.get_activation_tables
        bacc_mod.get_activation_tables = _patched_tables(orig)
        try:
            nc.compile()
        finally:
            bacc_mod.get_activation_tables = orig
    return nc


def _get_program():
    if "nc" not in _CACHE:
        _CACHE["nc"] = _build_program()
    return _CACHE["nc"]


def _prep_inputs(batch_titems, batch_citems, batch_pad_ids, t_emb, c_emb,
                 Ac_w, Ac_b, At_w, At_b, Bc_w, Bc_b, R_w, R_b):
    import ml_dtypes
    bf = ml_dtypes.bfloat16
    f = lambda x: np.ascontiguousarray(np.asarray(x, dtype=np.float32))
    temb = np.ascontiguousarray(np.asarray(t_emb, np.float32).astype(bf))
    cemb = np.ascontiguousarray(np.asarray(c_emb, np.float32).astype(bf))
    tit = np.asarray(batch_titems).astype(np.int32)
    cit = np.asarray(batch_citems).astype(np.int32)
    pad = np.asarray(batch_pad_ids).astype(np.int64)

    mask = np.zeros((B, M), np.float32)
    mask[pad[0], pad[1]] = NEG

    At_w = f(At_w); Ac_w = f(Ac_w); Bc_w = f(Bc_w); R_w = f(R_w)
    At_b = f(At_b); Ac_b = f(Ac_b); Bc_b = f(Bc_b); R_b = f(R_b)

    atw = np.zeros((128, 64), np.float32)
    atw[:, 0:D] = At_w.T
    acw = np.zeros((128, 64), np.float32)
    acw[:, 0:D] = Ac_w.T
    w2T = np.ascontiguousarray((R_w @ Bc_w).T)

    atb = np.zeros((128, 1), np.float32)
    atb[0:D, 0] = At_b
    atb[64:64 + D, 0] = At_b
    acb = np.zeros((128, 1), np.float32)
    acb[0:D, 0] = Ac_b
    acb[64:64 + D, 0] = Ac_b

    rbeff = (R_b + R_w @ Bc_b).astype(np.float32)

    eye = np.eye(128, dtype=np.float32)

    atw_b = atw.astype(bf); acw_b = acw.astype(bf)
    w2T_b = w2T.astype(bf); eye_b = eye.astype(bf)

    in_maps = []
    for c in range(NCORES):
        s = c * BLOC
        # v offsets: tight-packed [128, 51]; col c = flat rows 128c..128c+127
        vflat = np.zeros(6528, np.int64)
        vflat[0:6464] = tit[s : s + BLOC].reshape(-1)
        offt = np.ascontiguousarray(vflat.reshape(51, 128).T).astype(np.int32)
        # u offsets: tight-packed [128, 25]; col c = flat rows 128c..128c+127
        uflat = cit[s : s + BLOC].reshape(-1)  # [3200]
        offc = np.ascontiguousarray(uflat.reshape(25, 128).T).astype(np.int32)
        # mask: [128, 64] col b rows 0:50 = mask[b], rows 50:64 = NEG (pads)
        mk = np.full((128, BLOC), NEG, np.float32)
        mk[0:M, :] = mask[s : s + BLOC].T

        in_maps.append({
            "temb": temb, "cemb": cemb,
            "atw": atw_b, "acw": acw_b, "w2T": w2T_b,
            "atb": atb, "acb": acb, "eye": eye_b,
            "offt": offt, "offc": offc, "maskT": mk,
        })
    return in_maps, rbeff


def run_sharded(in_maps, **kwargs):
    from concourse.bass_utils import run_bass_kernel_spmd

    nc = _get_program()
    res = run_bass_kernel_spmd(nc, in_maps, core_ids=list(range(NCORES)), **kwargs)
    outs = [np.asarray(res.results[c]["out"]).transpose(1, 0, 2).astype(np.float32)
            for c in range(NCORES)]
    full = np.concatenate(outs, axis=0)
    return full, res


def kernel(**inputs):
    in_maps, rbeff = _prep_inputs(**inputs)
    full, _ = run_sharded(in_maps)
    return (full + rbeff[None, None, :]).astype(np.float32)
